# revision 13
# baseline (speedup 1.0000x reference)
"""GQA attention with 2D RoPE on 8 TRN2 NeuronCores — v2.

Sharding: batch data-parallel x4  X  head-group tensor-parallel x2.
Core c handles batch b=c//2 and head group g=c%2 (16 Q heads, 4 KV heads).
wo is row-sharded; partials are ReduceScattered per 512-col block across
each core pair, so core 2b returns rows 0:448 and core 2b+1 rows 448:896.

v2 vs v1 (same math):
  - merged DMAs (multi-dim APs): x in 4, wkv in 2, wq in 2, wo in 1,
    rec-bounces batched per head, cc_in / y path per-oc merges.
  - engine rebalance: masks on DVE, AT-muls on gpsimd(Pool), V evicts on
    Pool, K/Q psum evicts on DVE, phase-3 psum evicts + y-cast on Act.
  - issue-order software pipelining: per head scores groups lead AV groups
    by ~2 units to hide exp latency; Q og=1 projection quarters zipped
    between attention head pairs to keep PE fed while Act drains exps.

PSUM (8 banks): ph1a: psv 4 (2 b_ packed/bank) + psk 4.  ph1b: psq 8.
ph2: pss 2 tags x [128,4,256] (2 banks each) + psav 2 x [128,224] (1 bank)
+ psq1 2 x [128,448] (1 bank).  ph3: pso 7 x [128,512].
"""

import math
import numpy as np

import concourse.bass as bass
import concourse.tile as tile
import concourse.mybir as mybir
from concourse import bacc
from concourse import bass_utils

F32 = mybir.dt.float32
BF16 = mybir.dt.bfloat16
I32 = mybir.dt.int32
AF = mybir.ActivationFunctionType
ALU = mybir.AluOpType

B, L, D = 4, 896, 2048
HQ, HKV, HD = 32, 8, 64
NCORES = 8
GO = D // 2          # 1024 q-out dims per core
KVO = HKV * HD // 2  # 256 kv-out dims per core
NH = 16              # q heads per core
NKV = 4              # kv heads per core
P = 128
NI = D // P          # 16 contraction chunks
LB = L // P          # 7 key/l blocks
QCN = 4              # q chunks
QCW = L // QCN       # 224 q-chunk width
NKB = [2, 4, 6, 7]   # key blocks per q chunk (causal)
ROPE_BASE = 10000.0
TWO_PI = 2.0 * math.pi

# (qc, kb) pairs needing a causal mask, with affine_select base = 224*qc - 128*kb
PARTIAL = {}
for _qc in range(QCN):
    for _kb in range(NKB[_qc]):
        lo_key, hi_key = 128 * _kb, 128 * _kb + 127
        lo_row, hi_row = QCW * _qc, QCW * (_qc + 1) - 1
        if hi_key > lo_row:  # some key exceeds some row -> partial
            PARTIAL[(_qc, _kb)] = QCW * _qc - 128 * _kb

_NC_CACHE = {}


def build_nc(with_collective=True):
    key = with_collective
    if key in _NC_CACHE:
        return _NC_CACHE[key]
    nc = bacc.Bacc("TRN2", target_bir_lowering=False, debug=False,
                   num_devices=NCORES)
    ins = {
        "xT": nc.dram_tensor("xT", [D, L], BF16, kind="ExternalInput").ap(),
        "wqT": nc.dram_tensor("wqT", [D, GO], BF16, kind="ExternalInput").ap(),
        "wkvT": nc.dram_tensor("wkvT", [D, 2 * KVO], BF16,
                               kind="ExternalInput").ap(),
        "woT": nc.dram_tensor("woT", [GO, D], BF16, kind="ExternalInput").ap(),
        "Ct": nc.dram_tensor("Ct", [P, L], BF16, kind="ExternalInput").ap(),
        "St": nc.dram_tensor("St", [P, L], BF16, kind="ExternalInput").ap(),
    }
    y = nc.dram_tensor("y", [L // 2, D], F32, kind="ExternalOutput").ap()
    with tile.TileContext(nc) as tc:
        _build_kernel(nc, tc, ins, y, with_collective)
    nc.compile()
    _NC_CACHE[key] = nc
    return nc


def _ap3(dram_ap, row0, nrow_groups, group, ncols):
    """[128, nrow_groups, ncols] AP over dram rows row0.. in groups of 128.

    (p, c, f) -> row (row0 + 128*c + p), col f.
    """
    return bass.AP(tensor=dram_ap.tensor,
                   offset=dram_ap.offset + row0 * ncols,
                   ap=[[ncols, P], [group, nrow_groups], [1, ncols]])


def _bcast_row(dram_ap, parts, n):
    return bass.AP(tensor=dram_ap.tensor, offset=dram_ap.offset,
                   ap=[[0, parts], [1, n]])


def _rope(nc, pool, t, C, S):
    """t = t*C + shuffle16(t)*S, fully in place."""
    shuf = pool.tile([P, L], BF16, tag="rope_shuf")
    mask = [(p ^ 16) for p in range(32)]
    nc.vector.stream_shuffle(shuf[:], t[:], mask)
    nc.vector.tensor_mul(t[:], t[:], C[:])
    nc.vector.tensor_mul(shuf[:], shuf[:], S[:])
    nc.vector.tensor_add(t[:], t[:], shuf[:])


def _build_kernel(nc, tc, ins, y, with_collective):
    import contextlib
    ctx = contextlib.ExitStack()
    with ctx:
        const = ctx.enter_context(tc.tile_pool(name="const", bufs=1))

        # -------- persistent (whole-kernel) activation storage ----------
        big = ctx.enter_context(tc.tile_pool(name="big", bufs=1))
        WOB = big.tile([P, 8, D], BF16, tag="wob", name="wob")
        QT = [big.tile([P, L], BF16, tag=f"qt{i}", name=f"qt{i}")
              for i in range(8)]
        KTd = [big.tile([P, L], BF16, tag=f"kt{i}", name=f"kt{i}")
               for i in range(NKV)]
        # Vext[kv][kb][variant]: variant 0 = [V|1], 1 = [1|V]
        Vext = [[[big.tile([P, P], BF16, tag=f"v{k}_{b_}_{vr}",
                           name=f"v{k}_{b_}_{vr}")
                  for vr in range(2)] for b_ in range(LB)] for k in range(NKV)]
        AT = [big.tile([P, L], BF16, tag=f"at{i}", name=f"at{i}")
              for i in range(8)]

        # attention-phase pools (opened before proj so proj can pop first)
        ev = ctx.enter_context(tc.tile_pool(name="ev", bufs=2))
        upool = ctx.enter_context(tc.tile_pool(name="uatt", bufs=2))
        recpool = ctx.enter_context(tc.tile_pool(name="rec", bufs=2))
        # proj pool: closed after attention so phase 3 can reuse the space
        proj = tc.tile_pool(name="proj", bufs=1)
        prj = proj.__enter__()
        XT = prj.tile([P, NI, L], BF16, tag="xt", name="xt")
        WKV = prj.tile([P, NI, 2 * KVO], BF16, tag="wkv", name="wkv")
        WQ = prj.tile([P, NI, GO], BF16, tag="wq", name="wq")

        # ------------- rope tables (host-precomputed) + causal masks ----
        C = const.tile([P, L], BF16, tag="C", name="C")
        S = const.tile([P, L], BF16, tag="S", name="S")
        nc.sync.dma_start(C[:], ins["Ct"])
        nc.sync.dma_start(S[:], ins["St"])
        masks = {}
        for (qc, kb), base in PARTIAL.items():
            m = const.tile([P, QCW], BF16, tag=f"mask{qc}_{kb}",
                           name=f"mask{qc}_{kb}")
            nc.vector.memset(m[:], 1.0)
            nc.gpsimd.affine_select(out=m[:], in_=m[:], compare_op=ALU.is_ge,
                                    fill=0.0, base=base, channel_multiplier=-1,
                                    pattern=[[1, QCW]])
            masks[(qc, kb)] = m

        # ------------- input DMAs (SP queue, issue order = priority) ----
        nc.sync.dma_start(XT[:, 0:1, :], _ap3(ins["xT"], 0, 1, P * L, L))
        nc.sync.dma_start(WKV[:, 0:1, :],
                          _ap3(ins["wkvT"], 0, 1, P * 2 * KVO, 2 * KVO))
        nc.sync.dma_start(XT[:, 1:2, :], _ap3(ins["xT"], 128, 1, P * L, L))
        nc.sync.dma_start(WKV[:, 1:4, :],
                          _ap3(ins["wkvT"], 128, 3, P * 2 * KVO, 2 * KVO))
        for c8 in range(1, 8):  # rest of x in 2-chunk pieces, wkv interleaved
            nc.sync.dma_start(XT[:, 2 * c8:2 * c8 + 2, :],
                              _ap3(ins["xT"], 256 * c8, 2, P * L, L))
            if c8 < 4:
                nc.sync.dma_start(
                    WKV[:, 4 * c8:4 * c8 + 4, :],
                    _ap3(ins["wkvT"], 512 * c8, 4, P * 2 * KVO, 2 * KVO))
        for c2 in range(2):   # wq in 2 DMAs of 8 chunks
            nc.sync.dma_start(WQ[:, 8 * c2:8 * c2 + 8, :],
                              _ap3(ins["wqT"], 1024 * c2, 8, P * GO, GO))
        # ones halves of Vext (Pool; idle at start)
        for k in range(NKV):
            for b_ in range(LB):
                nc.gpsimd.memset(Vext[k][b_][0][:, 64:128], 1.0)
                nc.gpsimd.memset(Vext[k][b_][1][:, 0:64], 1.0)

        def qcopy(psq_a, psq_b, qt):
            nc.scalar.copy(qt[0:P, 0:448], psq_a[:])
            nc.scalar.copy(qt[0:P, 448:896], psq_b[:])

        def og0_half(psq_pool, half, tagbase):
            psq = [psq_pool.tile([P, 448], F32, tag=f"{tagbase}{j}",
                                 name=f"q0_{half}_{j}") for j in range(4)]
            for i in range(NI):
                for obh in range(2):
                    ob = half * 2 + obh
                    for h2 in range(2):
                        nc.tensor.matmul(
                            psq[obh * 2 + h2][:],
                            WQ[:, i, ob * P:(ob + 1) * P],
                            XT[:, i, h2 * 448:(h2 + 1) * 448],
                            start=(i == 0), stop=(i == NI - 1))
            return psq

        # ---------------- phase 1: V -> K -> Q og0, staged bank reuse ---
        with tc.tile_pool(name="ps1", bufs=1, space="PSUM") as ps1:
            psv = [ps1.tile([P, KVO], F32, tag=f"p{j}", name=f"pv{j}")
                   for j in range(LB)]
            for i in range(NI):
                st, sp = (i == 0), (i == NI - 1)
                for b_ in range(LB):
                    nc.tensor.matmul(
                        psv[b_][:],
                        XT[:, i, b_ * P:(b_ + 1) * P],
                        WKV[:, i, KVO:2 * KVO], start=st, stop=sp)
            # V evict (Act): psum -> Vext variants; b_ 0-3 first (K reuses)
            for b_ in range(LB):
                for k in range(NKV):
                    sl = psv[b_][:, k * 64:(k + 1) * 64]
                    nc.scalar.copy(Vext[k][b_][0][:, 0:64], sl)
                    nc.scalar.copy(Vext[k][b_][1][:, 64:128], sl)

            psk = [ps1.tile([P, 448], F32, tag=f"p{j}", name=f"pk{j}")
                   for j in range(4)]
            for i in range(NI):
                st, sp = (i == 0), (i == NI - 1)
                for ob in range(2):
                    for h2 in range(2):
                        nc.tensor.matmul(
                            psk[ob * 2 + h2][:],
                            WKV[:, i, ob * P:(ob + 1) * P],
                            XT[:, i, h2 * 448:(h2 + 1) * 448],
                            start=st, stop=sp)
            # K evict (Act) + rope (DVE) + duplicate into KTd halves
            for ob in range(2):
                roped = ev.tile([P, L], BF16, tag="roped")
                for h2 in range(2):
                    nc.scalar.copy(roped[:, h2 * 448:(h2 + 1) * 448],
                                   psk[ob * 2 + h2][:])
                _rope(nc, ev, roped, C, S)
                for sub in range(2):
                    k = ob * 2 + sub
                    src = roped[sub * 64:(sub + 1) * 64, :]
                    nc.sync.dma_start(KTd[k][0:64, :], src)
                    nc.sync.dma_start(KTd[k][64:128, :], src)

            def og0_half(tags, half):
                psq = [ps1.tile([P, 448], F32, tag=f"p{t}",
                                name=f"q0_{half}_{j}")
                       for j, t in enumerate(tags)]
                for i in range(NI):
                    for obh in range(2):
                        ob = half * 2 + obh
                        for h2 in range(2):
                            nc.tensor.matmul(
                                psq[obh * 2 + h2][:],
                                WQ[:, i, ob * P:(ob + 1) * P],
                                XT[:, i, h2 * 448:(h2 + 1) * 448],
                                start=(i == 0), stop=(i == NI - 1))
                return psq

            def qcopy(psq_a, psq_b, qt):
                nc.scalar.copy(qt[0:P, 0:448], psq_a[:])
                nc.scalar.copy(qt[0:P, 448:896], psq_b[:])

            # og0a on banks 4-6 + a fresh one (free after V evicts b_ 4-6)
            psq = og0_half((4, 5, 6, 7), 0)
            qcopy(psq[0], psq[1], QT[0])
            qcopy(psq[2], psq[3], QT[1])
            _rope(nc, ev, QT[0], C, S)     # runs on DVE during og0b
            _rope(nc, ev, QT[1], C, S)
            psqb = og0_half((0, 1, 2, 3), 1)   # K banks (evicted during og0a)
            qcopy(psqb[0], psqb[1], QT[2])
            qcopy(psqb[2], psqb[3], QT[3])
            _rope(nc, ev, QT[2], C, S)
            _rope(nc, ev, QT[3], C, S)

        # ---------------- phase 2 + zipped Q og=1 -----------------------
        pss_cm = tc.tile_pool(name="pss", bufs=1, space="PSUM")
        pss = pss_cm.__enter__()
        psav_cm = tc.tile_pool(name="psav", bufs=1, space="PSUM")
        psav = psav_cm.__enter__()
        s_ctr = [0]   # global score-slot rotation (shared with og1 eighths)

        def s_tile(name):
            t = pss.tile([P, 4, 256], F32, tag=f"s{s_ctr[0] % 2}", name=name)
            s_ctr[0] += 1
            return t

        def og1_eighth(j):
            """One (ob, h2) og=1 accumulation -> immediate Pool evict."""
            ob, h2 = j // 2, j % 2
            pq = pss.tile([P, 448], F32, tag=f"s{s_ctr[0] % 2}",
                          name=f"q1_{j}")
            s_ctr[0] += 1
            for i in range(NI):
                nc.tensor.matmul(
                    pq[:],
                    WQ[:, i, 512 + ob * P:512 + (ob + 1) * P],
                    XT[:, i, h2 * 448:(h2 + 1) * 448],
                    start=(i == 0), stop=(i == NI - 1))
            nc.vector.tensor_copy(QT[4 + ob][:, h2 * 448:(h2 + 1) * 448],
                                  pq[:])
            if h2 == 1:
                _rope(nc, ev, QT[4 + ob], C, S)

        def attention_head(h):
            kv = h // 4
            qblk, qsub = divmod(h, 2)
            qoff = qsub * 64
            soff = 64 - qoff
            vr = qsub
            # flat score/exp groups: (qc, k0, ng, U)
            groups = []
            for qc in range(QCN):
                nkb = NKB[qc]
                for k0 in range(0, nkb, 4):
                    groups.append((qc, k0, min(4, nkb - k0)))
            # AV psum: qc pairs packed 2-per-bank so only 2 tags are live
            av_tiles = {}
            recs = recpool.tile([P, QCN, QCW], F32, tag="recs", bufs=1,
                                name=f"recs{h}")
            done_u = {}

            def emit_scores(gi):
                qc, k0, ng = groups[gi]
                qsl = slice(qc * QCW, (qc + 1) * QCW)
                ps_s = s_tile(f"s{h}_{qc}_{k0}")
                for j in range(ng):
                    nc.tensor.matmul(
                        ps_s[:, j, 0:QCW],
                        KTd[kv][qoff:qoff + 64,
                                (k0 + j) * P:(k0 + j + 1) * P],
                        QT[qblk][qoff:qoff + 64, qsl],
                        start=True, stop=True, tile_position=(qoff, 0))
                U = upool.tile([P, 4, QCW], BF16, tag=f"u{gi % 2}",
                               name=f"u{h}_{qc}_{k0}")
                nc.scalar.activation(U[:, 0:ng, 0:QCW], ps_s[:, 0:ng, 0:QCW],
                                     AF.Exp, scale=0.125)
                for j in range(ng):
                    if (qc, k0 + j) in PARTIAL:
                        eng = nc.gpsimd if (h + j) % 2 else nc.vector
                        eng.tensor_tensor(
                            U[:, j, 0:QCW], U[:, j, 0:QCW],
                            masks[(qc, k0 + j)][:], op=ALU.mult)
                done_u[(qc, k0)] = (ng, U)

            def emit_av(qc):
                nkb = NKB[qc]
                if qc % 2 == 0:
                    av_tiles[qc // 2] = psav.tile(
                        [P, 2, QCW], F32, tag=f"av{qc // 2}", bufs=2,
                        name=f"av{h}_{qc // 2}")
                ps_av = av_tiles[qc // 2][:, qc % 2, :]
                kb = 0
                for k0 in range(0, nkb, 4):
                    ng, U = done_u[(qc, k0)]
                    for j in range(ng):
                        nc.tensor.matmul(
                            ps_av, Vext[kv][k0 + j][vr][:],
                            U[:, j, 0:QCW],
                            start=(kb == 0), stop=(kb == nkb - 1))
                        kb += 1
                if qc % 2 == 1:   # one reciprocal per qc pair
                    nc.vector.reciprocal(
                        recs[soff:soff + 64, qc - 1:qc + 1, :],
                        av_tiles[qc // 2][soff:soff + 64, :, :])

            # interleave: scores lead AV by ~2 groups
            order = [("s", 0), ("s", 1), ("s", 2), ("av", 0), ("s", 3),
                     ("av", 1), ("s", 4), ("av", 2), ("s", 5), ("av", 3)]
            for kind, idx in order:
                if kind == "s":
                    emit_scores(idx)
                else:
                    emit_av(idx)

            # one partition-shift DMA for all 4 reciprocals of this head
            rec = recpool.tile([P, QCN, QCW], F32, tag="rec", bufs=1,
                                name=f"rec{h}")
            nc.sync.dma_start(rec[qoff:qoff + 64, :, :],
                              recs[soff:soff + 64, :, :])
            # AT writes (DVE), one op per qc pair
            for c in range(2):
                qsl = slice(2 * c * QCW, 2 * (c + 1) * QCW)
                nc.vector.tensor_tensor(
                    AT[qblk][qoff:qoff + 64, qsl],
                    av_tiles[c][qoff:qoff + 64, :, :],
                    rec[qoff:qoff + 64, 2 * c:2 * c + 2, :], op=ALU.mult)

        for h in range(8):
            attention_head(h)
            og1_eighth(h)
            if h in (2, 3, 4, 5):   # wo load mid-attention, 4 pieces
                c = h - 2
                nc.sync.dma_start(WOB[:, 2 * c:2 * c + 2, :],
                                  _ap3(ins["woT"], 256 * c, 2, P * D, D))
        for h in range(8, 16):
            attention_head(h)
        psav_cm.__exit__(None, None, None)
        pss_cm.__exit__(None, None, None)
        proj.__exit__(None, None, None)

        # ---------------- phase 3: out projection + reduce-scatter -------
        with tc.tile_pool(name="osb", bufs=2) as osb, \
             tc.tile_pool(name="pso", bufs=1, space="PSUM") as pso, \
             tc.tile_pool(name="ccdram", bufs=1, space="DRAM") as ccdram:
            cc_in = [ccdram.tile([L, 512], BF16, tag=f"ccin{oc}",
                                 name=f"ccin{oc}") for oc in range(4)]
            cc_out = [ccdram.tile([L // 2, 512], BF16, tag=f"ccout{oc}",
                                  name=f"ccout{oc}") for oc in range(4)]
            for oc in range(4):
                pso_t = [pso.tile([P, 512], F32, tag=f"po{b_}",
                                  name=f"pso{oc}_{b_}") for b_ in range(LB)]
                border = ([0, 1, 2, 3], [4, 5, 6]) if oc == 0 else (list(range(LB)),)
                for grp in border:
                    for ic in range(8):
                        for b_ in grp:
                            nc.tensor.matmul(
                                pso_t[b_][:], AT[ic][:, b_ * P:(b_ + 1) * P],
                                WOB[:, ic, oc * 512:(oc + 1) * 512],
                                start=(ic == 0), stop=(ic == 7))
                ot = osb.tile([P, LB, 512], BF16, tag="ot", name=f"ot{oc}")
                for b_ in range(LB):
                    nc.scalar.copy(ot[:, b_, :], pso_t[b_][:])
                    nc.sync.dma_start(
                        bass.AP(tensor=cc_in[oc].tensor,
                                offset=cc_in[oc].offset + b_ * P * 512,
                                ap=[[512, P], [1, 512]]),
                        ot[:, b_, :])
                src_dram = cc_out[oc]
                if with_collective:
                    nc.gpsimd.collective_compute(
                        "ReduceScatter", ALU.add,
                        replica_groups=[[0, 1], [2, 3], [4, 5], [6, 7]],
                        ins=[cc_in[oc].opt()], outs=[cc_out[oc].opt()])
                else:
                    src_dram = cc_in[oc]
                # bf16 -> f32 via SBUF bounce (no casting DMAs)
                for r0, rg in ((0, 3), (384, 1)):
                    rn = P if rg == 3 else 64
                    yb = osb.tile([P, 3, 512], BF16, tag="yb",
                                  name=f"yb{oc}_{r0}")
                    nc.sync.dma_start(
                        yb[0:rn, 0:rg, :],
                        bass.AP(tensor=src_dram.tensor,
                                offset=src_dram.offset + r0 * 512,
                                ap=[[512, rn], [512 * P, rg], [1, 512]]))
                    yf = osb.tile([P, 3, 512], F32, tag="yf",
                                  name=f"yf{oc}_{r0}")
                    nc.scalar.copy(yf[0:rn, 0:rg, :], yb[0:rn, 0:rg, :])
                    nc.sync.dma_start(
                        bass.AP(tensor=y.tensor,
                                offset=y.offset + r0 * D + oc * 512,
                                ap=[[D, rn], [D * P, rg], [1, 512]]),
                        yf[0:rn, 0:rg, :])


# ---------------------------------------------------------------- host side
_ROPE_PERM = np.concatenate([
    np.arange(0, 32, 2), np.arange(1, 32, 2),
    np.arange(32, 64, 2), np.arange(33, 64, 2)])


def make_in_maps(x, wq, wk, wv, wo, temporal_pos, structural_pos):
    import ml_dtypes
    bf16 = ml_dtypes.bfloat16
    x = np.asarray(x, dtype=np.float32)
    wq = np.asarray(wq, dtype=np.float32)
    wk = np.asarray(wk, dtype=np.float32)
    wv = np.asarray(wv, dtype=np.float32)
    wo = np.asarray(wo, dtype=np.float32)
    pt = np.asarray(temporal_pos).astype(np.float64)
    ps = np.asarray(structural_pos).astype(np.float64)
    inv = 1.0 / (10000.0 ** (np.arange(16) / 16.0))
    ct, st = np.cos(pt[:, None] * inv).T, np.sin(pt[:, None] * inv).T
    cs, ss = np.cos(ps[:, None] * inv).T, np.sin(ps[:, None] * inv).T
    Ct = np.concatenate([ct, ct, cs, cs] * 2).astype(bf16)     # [128, 896]
    St = np.concatenate([-st, st, -ss, ss] * 2).astype(bf16)

    wq_p = wq.reshape(HQ, HD, D)[:, _ROPE_PERM, :].reshape(D, D)
    wk_p = wk.reshape(HKV, HD, D)[:, _ROPE_PERM, :].reshape(HKV * HD, D)
    wqT = np.ascontiguousarray(wq_p.T).astype(bf16)   # [D, D]
    wkT = np.ascontiguousarray(wk_p.T).astype(bf16)   # [D, 512]
    wvT = np.ascontiguousarray(wv.T).astype(bf16)     # [D, 512]
    woT = np.ascontiguousarray(wo.T).astype(bf16)     # [D, D]

    in_maps = []
    for c in range(NCORES):
        b, g = divmod(c, 2)
        wkv = np.concatenate([wkT[:, g * KVO:(g + 1) * KVO],
                              wvT[:, g * KVO:(g + 1) * KVO]], axis=1)
        in_maps.append({
            "xT": np.ascontiguousarray(x[b].T).astype(bf16),
            "wqT": np.ascontiguousarray(wqT[:, g * GO:(g + 1) * GO]),
            "wkvT": np.ascontiguousarray(wkv),
            "woT": np.ascontiguousarray(woT[g * GO:(g + 1) * GO, :]),
            "Ct": Ct,
            "St": St,
        })
    return in_maps


def kernel(x, wq, wk, wv, wo, temporal_pos, structural_pos, _trace=False):
    nc = build_nc(with_collective=True)
    in_maps = make_in_maps(x, wq, wk, wv, wo, temporal_pos, structural_pos)
    res = bass_utils.run_bass_kernel_spmd(
        nc, in_maps, core_ids=list(range(NCORES)), trace=_trace)
    out = np.stack([
        np.concatenate([res.results[2 * b]["y"], res.results[2 * b + 1]["y"]],
                       axis=0) for b in range(B)])
    kernel.last_result = res
    return out.astype(np.float32)


# revision 15
# speedup vs baseline: 1.0026x; 1.0026x over previous
"""GQA attention with 2D RoPE on 8 TRN2 NeuronCores — v2.

Sharding: batch data-parallel x4  X  head-group tensor-parallel x2.
Core c handles batch b=c//2 and head group g=c%2 (16 Q heads, 4 KV heads).
wo is row-sharded; partials are ReduceScattered per 512-col block across
each core pair, so core 2b returns rows 0:448 and core 2b+1 rows 448:896.

v2 vs v1 (same math):
  - merged DMAs (multi-dim APs): x in 4, wkv in 2, wq in 2, wo in 1,
    rec-bounces batched per head, cc_in / y path per-oc merges.
  - engine rebalance: masks on DVE, AT-muls on gpsimd(Pool), V evicts on
    Pool, K/Q psum evicts on DVE, phase-3 psum evicts + y-cast on Act.
  - issue-order software pipelining: per head scores groups lead AV groups
    by ~2 units to hide exp latency; Q og=1 projection quarters zipped
    between attention head pairs to keep PE fed while Act drains exps.

PSUM (8 banks): ph1a: psv 4 (2 b_ packed/bank) + psk 4.  ph1b: psq 8.
ph2: pss 2 tags x [128,4,256] (2 banks each) + psav 2 x [128,224] (1 bank)
+ psq1 2 x [128,448] (1 bank).  ph3: pso 7 x [128,512].
"""

import math
import numpy as np

import concourse.bass as bass
import concourse.tile as tile
import concourse.mybir as mybir
from concourse import bacc
from concourse import bass_utils

F32 = mybir.dt.float32
BF16 = mybir.dt.bfloat16
I32 = mybir.dt.int32
AF = mybir.ActivationFunctionType
ALU = mybir.AluOpType

B, L, D = 4, 896, 2048
HQ, HKV, HD = 32, 8, 64
NCORES = 8
GO = D // 2          # 1024 q-out dims per core
KVO = HKV * HD // 2  # 256 kv-out dims per core
NH = 16              # q heads per core
NKV = 4              # kv heads per core
P = 128
NI = D // P          # 16 contraction chunks
LB = L // P          # 7 key/l blocks
QCN = 4              # q chunks
QCW = L // QCN       # 224 q-chunk width
NKB = [2, 4, 6, 7]   # key blocks per q chunk (causal)
ROPE_BASE = 10000.0
TWO_PI = 2.0 * math.pi

# (qc, kb) pairs needing a causal mask, with affine_select base = 224*qc - 128*kb
PARTIAL = {}
for _qc in range(QCN):
    for _kb in range(NKB[_qc]):
        lo_key, hi_key = 128 * _kb, 128 * _kb + 127
        lo_row, hi_row = QCW * _qc, QCW * (_qc + 1) - 1
        if hi_key > lo_row:  # some key exceeds some row -> partial
            PARTIAL[(_qc, _kb)] = QCW * _qc - 128 * _kb

_NC_CACHE = {}


def build_nc(with_collective=True):
    key = with_collective
    if key in _NC_CACHE:
        return _NC_CACHE[key]
    nc = bacc.Bacc("TRN2", target_bir_lowering=False, debug=False,
                   num_devices=NCORES)
    ins = {
        "xT": nc.dram_tensor("xT", [D, L], BF16, kind="ExternalInput").ap(),
        "wqT": nc.dram_tensor("wqT", [D, GO], BF16, kind="ExternalInput").ap(),
        "wkvT": nc.dram_tensor("wkvT", [D, 2 * KVO], BF16,
                               kind="ExternalInput").ap(),
        "woT": nc.dram_tensor("woT", [GO, D], BF16, kind="ExternalInput").ap(),
        "Ct": nc.dram_tensor("Ct", [P, L], BF16, kind="ExternalInput").ap(),
        "St": nc.dram_tensor("St", [P, L], BF16, kind="ExternalInput").ap(),
    }
    y = nc.dram_tensor("y", [L // 2, D], F32, kind="ExternalOutput").ap()
    with tile.TileContext(nc) as tc:
        _build_kernel(nc, tc, ins, y, with_collective)
    nc.compile()
    _NC_CACHE[key] = nc
    return nc


def _ap3(dram_ap, row0, nrow_groups, group, ncols):
    """[128, nrow_groups, ncols] AP over dram rows row0.. in groups of 128.

    (p, c, f) -> row (row0 + 128*c + p), col f.
    """
    return bass.AP(tensor=dram_ap.tensor,
                   offset=dram_ap.offset + row0 * ncols,
                   ap=[[ncols, P], [group, nrow_groups], [1, ncols]])


def _bcast_row(dram_ap, parts, n):
    return bass.AP(tensor=dram_ap.tensor, offset=dram_ap.offset,
                   ap=[[0, parts], [1, n]])


def _rope(nc, pool, t, C, S):
    """t = t*C + shuffle16(t)*S, fully in place."""
    shuf = pool.tile([P, L], BF16, tag="rope_shuf")
    mask = [(p ^ 16) for p in range(32)]
    nc.vector.stream_shuffle(shuf[:], t[:], mask)
    nc.vector.tensor_mul(t[:], t[:], C[:])
    nc.vector.tensor_mul(shuf[:], shuf[:], S[:])
    nc.vector.tensor_add(t[:], t[:], shuf[:])


def _build_kernel(nc, tc, ins, y, with_collective):
    import contextlib
    ctx = contextlib.ExitStack()
    with ctx:
        const = ctx.enter_context(tc.tile_pool(name="const", bufs=1))

        # -------- persistent (whole-kernel) activation storage ----------
        big = ctx.enter_context(tc.tile_pool(name="big", bufs=1))
        WOB = big.tile([P, 8, D], BF16, tag="wob", name="wob")
        QT = [big.tile([P, L], BF16, tag=f"qt{i}", name=f"qt{i}")
              for i in range(8)]
        KTd = [big.tile([P, L], BF16, tag=f"kt{i}", name=f"kt{i}")
               for i in range(NKV)]
        # Vext[kv][kb][variant]: variant 0 = [V|1], 1 = [1|V]
        Vext = [[[big.tile([P, P], BF16, tag=f"v{k}_{b_}_{vr}",
                           name=f"v{k}_{b_}_{vr}")
                  for vr in range(2)] for b_ in range(LB)] for k in range(NKV)]
        AT = [big.tile([P, L], BF16, tag=f"at{i}", name=f"at{i}")
              for i in range(8)]

        # attention-phase pools (opened before proj so proj can pop first)
        ev = ctx.enter_context(tc.tile_pool(name="ev", bufs=2))
        upool = ctx.enter_context(tc.tile_pool(name="uatt", bufs=2))
        recpool = ctx.enter_context(tc.tile_pool(name="rec", bufs=2))
        # proj pool: closed after attention so phase 3 can reuse the space
        proj = tc.tile_pool(name="proj", bufs=1)
        prj = proj.__enter__()
        XT = prj.tile([P, NI, L], BF16, tag="xt", name="xt")
        WKV = prj.tile([P, NI, 2 * KVO], BF16, tag="wkv", name="wkv")
        WQ = prj.tile([P, NI, GO], BF16, tag="wq", name="wq")

        # ------------- rope tables (host-precomputed) + causal masks ----
        C = const.tile([P, L], BF16, tag="C", name="C")
        S = const.tile([P, L], BF16, tag="S", name="S")
        nc.sync.dma_start(C[:], ins["Ct"])
        nc.sync.dma_start(S[:], ins["St"])
        masks = {}
        for (qc, kb), base in PARTIAL.items():
            m = const.tile([P, QCW], BF16, tag=f"mask{qc}_{kb}",
                           name=f"mask{qc}_{kb}")
            nc.vector.memset(m[:], 1.0)
            nc.gpsimd.affine_select(out=m[:], in_=m[:], compare_op=ALU.is_ge,
                                    fill=0.0, base=base, channel_multiplier=-1,
                                    pattern=[[1, QCW]])
            masks[(qc, kb)] = m

        # ------------- input DMAs (SP queue, issue order = priority) ----
        nc.sync.dma_start(XT[:, 0:1, :], _ap3(ins["xT"], 0, 1, P * L, L))
        nc.sync.dma_start(WKV[:, 0:1, :],
                          _ap3(ins["wkvT"], 0, 1, P * 2 * KVO, 2 * KVO))
        nc.sync.dma_start(XT[:, 1:2, :], _ap3(ins["xT"], 128, 1, P * L, L))
        nc.sync.dma_start(WKV[:, 1:4, :],
                          _ap3(ins["wkvT"], 128, 3, P * 2 * KVO, 2 * KVO))
        for c8 in range(1, 8):  # rest of x in 2-chunk pieces, wkv interleaved
            nc.sync.dma_start(XT[:, 2 * c8:2 * c8 + 2, :],
                              _ap3(ins["xT"], 256 * c8, 2, P * L, L))
            if c8 < 4:
                nc.sync.dma_start(
                    WKV[:, 4 * c8:4 * c8 + 4, :],
                    _ap3(ins["wkvT"], 512 * c8, 4, P * 2 * KVO, 2 * KVO))
        for c2 in range(2):   # wq in 2 DMAs of 8 chunks
            nc.sync.dma_start(WQ[:, 8 * c2:8 * c2 + 8, :],
                              _ap3(ins["wqT"], 1024 * c2, 8, P * GO, GO))
        # ones halves of Vext (Pool; idle at start)
        for k in range(NKV):
            for b_ in range(LB):
                nc.gpsimd.memset(Vext[k][b_][0][:, 64:128], 1.0)
                nc.gpsimd.memset(Vext[k][b_][1][:, 0:64], 1.0)

        def qcopy(psq_a, psq_b, qt):
            nc.scalar.copy(qt[0:P, 0:448], psq_a[:])
            nc.scalar.copy(qt[0:P, 448:896], psq_b[:])

        def og0_half(psq_pool, half, tagbase):
            psq = [psq_pool.tile([P, 448], F32, tag=f"{tagbase}{j}",
                                 name=f"q0_{half}_{j}") for j in range(4)]
            for i in range(NI):
                for obh in range(2):
                    ob = half * 2 + obh
                    for h2 in range(2):
                        nc.tensor.matmul(
                            psq[obh * 2 + h2][:],
                            WQ[:, i, ob * P:(ob + 1) * P],
                            XT[:, i, h2 * 448:(h2 + 1) * 448],
                            start=(i == 0), stop=(i == NI - 1))
            return psq

        # ---------------- phase 1: V -> K -> Q og0, staged bank reuse ---
        with tc.tile_pool(name="ps1", bufs=1, space="PSUM") as ps1:
            psv = [ps1.tile([P, KVO], F32, tag=f"p{j}", name=f"pv{j}")
                   for j in range(LB)]
            for i in range(NI):
                st, sp = (i == 0), (i == NI - 1)
                for b_ in range(LB):
                    nc.tensor.matmul(
                        psv[b_][:],
                        XT[:, i, b_ * P:(b_ + 1) * P],
                        WKV[:, i, KVO:2 * KVO], start=st, stop=sp)
            # V evict (Act): psum -> Vext variants; b_ 0-3 first (K reuses)
            for b_ in range(LB):
                for k in range(NKV):
                    sl = psv[b_][:, k * 64:(k + 1) * 64]
                    nc.scalar.copy(Vext[k][b_][0][:, 0:64], sl)
                    nc.scalar.copy(Vext[k][b_][1][:, 64:128], sl)

            psk = [ps1.tile([P, 448], F32, tag=f"p{j}", name=f"pk{j}")
                   for j in range(4)]
            for i in range(NI):
                st, sp = (i == 0), (i == NI - 1)
                for ob in range(2):
                    for h2 in range(2):
                        nc.tensor.matmul(
                            psk[ob * 2 + h2][:],
                            WKV[:, i, ob * P:(ob + 1) * P],
                            XT[:, i, h2 * 448:(h2 + 1) * 448],
                            start=st, stop=sp)
            # K evict (Act) + rope (DVE) + duplicate into KTd halves
            for ob in range(2):
                roped = ev.tile([P, L], BF16, tag="roped")
                for h2 in range(2):
                    nc.scalar.copy(roped[:, h2 * 448:(h2 + 1) * 448],
                                   psk[ob * 2 + h2][:])
                _rope(nc, ev, roped, C, S)
                for sub in range(2):
                    k = ob * 2 + sub
                    src = roped[sub * 64:(sub + 1) * 64, :]
                    nc.sync.dma_start(KTd[k][0:64, :], src)
                    nc.sync.dma_start(KTd[k][64:128, :], src)

            def og0_half(tags, half):
                psq = [ps1.tile([P, 448], F32, tag=f"p{t}",
                                name=f"q0_{half}_{j}")
                       for j, t in enumerate(tags)]
                for i in range(NI):
                    for obh in range(2):
                        ob = half * 2 + obh
                        for h2 in range(2):
                            nc.tensor.matmul(
                                psq[obh * 2 + h2][:],
                                WQ[:, i, ob * P:(ob + 1) * P],
                                XT[:, i, h2 * 448:(h2 + 1) * 448],
                                start=(i == 0), stop=(i == NI - 1))
                return psq

            def qcopy(psq_a, psq_b, qt):
                nc.scalar.copy(qt[0:P, 0:448], psq_a[:])
                nc.scalar.copy(qt[0:P, 448:896], psq_b[:])

            # og0a on banks 4-6 + a fresh one (free after V evicts b_ 4-6)
            psq = og0_half((4, 5, 6, 7), 0)
            qcopy(psq[0], psq[1], QT[0])
            qcopy(psq[2], psq[3], QT[1])
            _rope(nc, ev, QT[0], C, S)     # runs on DVE during og0b
            _rope(nc, ev, QT[1], C, S)
            psqb = og0_half((0, 1, 2, 3), 1)   # K banks (evicted during og0a)
            qcopy(psqb[0], psqb[1], QT[2])
            qcopy(psqb[2], psqb[3], QT[3])
            _rope(nc, ev, QT[2], C, S)
            _rope(nc, ev, QT[3], C, S)

        # ---------------- phase 2 + zipped Q og=1 -----------------------
        pss_cm = tc.tile_pool(name="pss", bufs=1, space="PSUM")
        pss = pss_cm.__enter__()
        psav_cm = tc.tile_pool(name="psav", bufs=1, space="PSUM")
        psav = psav_cm.__enter__()
        s_ctr = [0]   # global score-slot rotation (shared with og1 eighths)

        def s_tile(name):
            t = pss.tile([P, 4, 256], F32, tag=f"s{s_ctr[0] % 2}", name=name)
            s_ctr[0] += 1
            return t

        def og1_eighth(j):
            """One (ob, h2) og=1 accumulation -> immediate Pool evict."""
            ob, h2 = j // 2, j % 2
            pq = pss.tile([P, 448], F32, tag=f"s{s_ctr[0] % 2}",
                          name=f"q1_{j}")
            s_ctr[0] += 1
            for i in range(NI):
                nc.tensor.matmul(
                    pq[:],
                    WQ[:, i, 512 + ob * P:512 + (ob + 1) * P],
                    XT[:, i, h2 * 448:(h2 + 1) * 448],
                    start=(i == 0), stop=(i == NI - 1))
            nc.vector.tensor_copy(QT[4 + ob][:, h2 * 448:(h2 + 1) * 448],
                                  pq[:])
            if h2 == 1:
                _rope(nc, ev, QT[4 + ob], C, S)

        def attention_head(h):
            kv = h // 4
            qblk, qsub = divmod(h, 2)
            qoff = qsub * 64
            soff = 64 - qoff
            vr = qsub
            # flat score/exp groups: (qc, k0, ng, U)
            groups = []
            for qc in range(QCN):
                nkb = NKB[qc]
                for k0 in range(0, nkb, 4):
                    groups.append((qc, k0, min(4, nkb - k0)))
            # AV psum: qc pairs packed 2-per-bank so only 2 tags are live
            av_tiles = {}
            recs = recpool.tile([P, QCN, QCW], F32, tag="recs", bufs=1,
                                name=f"recs{h}")
            done_u = {}

            def emit_scores(gi):
                qc, k0, ng = groups[gi]
                qsl = slice(qc * QCW, (qc + 1) * QCW)
                ps_s = s_tile(f"s{h}_{qc}_{k0}")
                for j in range(ng):
                    nc.tensor.matmul(
                        ps_s[:, j, 0:QCW],
                        KTd[kv][qoff:qoff + 64,
                                (k0 + j) * P:(k0 + j + 1) * P],
                        QT[qblk][qoff:qoff + 64, qsl],
                        start=True, stop=True, tile_position=(qoff, 0))
                U = upool.tile([P, 4, QCW], BF16, tag=f"u{gi % 2}",
                               name=f"u{h}_{qc}_{k0}")
                nc.scalar.activation(U[:, 0:ng, 0:QCW], ps_s[:, 0:ng, 0:QCW],
                                     AF.Exp, scale=0.125)
                for j in range(ng):
                    if (qc, k0 + j) in PARTIAL:
                        eng = nc.gpsimd if (h + j) % 2 else nc.vector
                        eng.tensor_tensor(
                            U[:, j, 0:QCW], U[:, j, 0:QCW],
                            masks[(qc, k0 + j)][:], op=ALU.mult)
                done_u[(qc, k0)] = (ng, U)

            def emit_av(qc):
                nkb = NKB[qc]
                if qc % 2 == 0:
                    av_tiles[qc // 2] = psav.tile(
                        [P, 2, QCW], F32, tag=f"av{qc // 2}", bufs=2,
                        name=f"av{h}_{qc // 2}")
                ps_av = av_tiles[qc // 2][:, qc % 2, :]
                kb = 0
                for k0 in range(0, nkb, 4):
                    ng, U = done_u[(qc, k0)]
                    for j in range(ng):
                        nc.tensor.matmul(
                            ps_av, Vext[kv][k0 + j][vr][:],
                            U[:, j, 0:QCW],
                            start=(kb == 0), stop=(kb == nkb - 1))
                        kb += 1
                if qc % 2 == 1:   # one reciprocal per qc pair
                    nc.vector.reciprocal(
                        recs[soff:soff + 64, qc - 1:qc + 1, :],
                        av_tiles[qc // 2][soff:soff + 64, :, :])

            # interleave: scores lead AV by ~2 groups
            order = [("s", 0), ("s", 1), ("s", 2), ("av", 0), ("s", 3),
                     ("av", 1), ("s", 4), ("av", 2), ("s", 5), ("av", 3)]
            for kind, idx in order:
                if kind == "s":
                    emit_scores(idx)
                else:
                    emit_av(idx)

            # one partition-shift DMA for all 4 reciprocals of this head
            rec = recpool.tile([P, QCN, QCW], F32, tag="rec", bufs=1,
                                name=f"rec{h}")
            nc.sync.dma_start(rec[qoff:qoff + 64, :, :],
                              recs[soff:soff + 64, :, :])
            # AT writes (DVE), one op per qc pair
            for c in range(2):
                qsl = slice(2 * c * QCW, 2 * (c + 1) * QCW)
                nc.vector.tensor_tensor(
                    AT[qblk][qoff:qoff + 64, qsl],
                    av_tiles[c][qoff:qoff + 64, :, :],
                    rec[qoff:qoff + 64, 2 * c:2 * c + 2, :], op=ALU.mult)

        for h in range(8):
            attention_head(h)
            og1_eighth(h)
            if h in (2, 3, 4, 5):   # wo load mid-attention, 4 pieces
                c = h - 2
                nc.sync.dma_start(WOB[:, 2 * c:2 * c + 2, :],
                                  _ap3(ins["woT"], 256 * c, 2, P * D, D))
        for h in range(8, 16):
            attention_head(h)
        psav_cm.__exit__(None, None, None)
        pss_cm.__exit__(None, None, None)
        proj.__exit__(None, None, None)

        # ---------------- phase 3: out projection + reduce-scatter -------
        # 2 collectives (15us fixed cost each in the model): one per oc pair
        with tc.tile_pool(name="osb", bufs=2) as osb, \
             tc.tile_pool(name="pso", bufs=1, space="PSUM") as pso, \
             tc.tile_pool(name="ccdram", bufs=1, space="DRAM") as ccdram:
            cc_in = [ccdram.tile([L, 1024], BF16, tag=f"ccin{g_}",
                                 name=f"ccin{g_}") for g_ in range(2)]
            cc_out = [ccdram.tile([L // 2, 1024], BF16, tag=f"ccout{g_}",
                                  name=f"ccout{g_}") for g_ in range(2)]
            for oc in range(4):
                g_, half = divmod(oc, 2)
                pso_t = [pso.tile([P, 512], F32, tag=f"po{b_}",
                                  name=f"pso{oc}_{b_}") for b_ in range(LB)]
                ot = osb.tile([P, LB, 512], BF16, tag="ot", name=f"ot{oc}")
                for b_ in range(LB):   # b_-outer: evict+send as soon as done
                    for ic in range(8):
                        nc.tensor.matmul(
                            pso_t[b_][:], AT[ic][:, b_ * P:(b_ + 1) * P],
                            WOB[:, ic, oc * 512:(oc + 1) * 512],
                            start=(ic == 0), stop=(ic == 7))
                    nc.scalar.copy(ot[:, b_, :], pso_t[b_][:])
                    nc.sync.dma_start(
                        bass.AP(tensor=cc_in[g_].tensor,
                                offset=(cc_in[g_].offset + b_ * P * 1024
                                        + half * 512),
                                ap=[[1024, P], [1, 512]]),
                        ot[:, b_, :])
                if half == 1:
                    src_dram = cc_out[g_]
                    if with_collective:
                        nc.gpsimd.collective_compute(
                            "ReduceScatter", ALU.add,
                            replica_groups=[[0, 1], [2, 3], [4, 5], [6, 7]],
                            ins=[cc_in[g_].opt()], outs=[cc_out[g_].opt()])
                    else:
                        src_dram = cc_in[g_]
                    # bf16 -> f32 via SBUF bounce, 128-row pipelined chunks
                    for r0 in (0, 128, 256, 384):
                        rn = 64 if r0 == 384 else P
                        yb = osb.tile([P, 1024], BF16, tag="yb",
                                      name=f"yb{g_}_{r0}")
                        nc.sync.dma_start(
                            yb[0:rn, :],
                            bass.AP(tensor=src_dram.tensor,
                                    offset=src_dram.offset + r0 * 1024,
                                    ap=[[1024, rn], [1, 1024]]))
                        yf = osb.tile([P, 1024], F32, tag="yf",
                                      name=f"yf{g_}_{r0}")
                        nc.scalar.copy(yf[0:rn, :], yb[0:rn, :])
                        nc.sync.dma_start(
                            bass.AP(tensor=y.tensor,
                                    offset=(y.offset + r0 * D + g_ * 1024),
                                    ap=[[D, rn], [1, 1024]]),
                            yf[0:rn, :])


# ---------------------------------------------------------------- host side
_ROPE_PERM = np.concatenate([
    np.arange(0, 32, 2), np.arange(1, 32, 2),
    np.arange(32, 64, 2), np.arange(33, 64, 2)])


def make_in_maps(x, wq, wk, wv, wo, temporal_pos, structural_pos):
    import ml_dtypes
    bf16 = ml_dtypes.bfloat16
    x = np.asarray(x, dtype=np.float32)
    wq = np.asarray(wq, dtype=np.float32)
    wk = np.asarray(wk, dtype=np.float32)
    wv = np.asarray(wv, dtype=np.float32)
    wo = np.asarray(wo, dtype=np.float32)
    pt = np.asarray(temporal_pos).astype(np.float64)
    ps = np.asarray(structural_pos).astype(np.float64)
    inv = 1.0 / (10000.0 ** (np.arange(16) / 16.0))
    ct, st = np.cos(pt[:, None] * inv).T, np.sin(pt[:, None] * inv).T
    cs, ss = np.cos(ps[:, None] * inv).T, np.sin(ps[:, None] * inv).T
    Ct = np.concatenate([ct, ct, cs, cs] * 2).astype(bf16)     # [128, 896]
    St = np.concatenate([-st, st, -ss, ss] * 2).astype(bf16)

    wq_p = wq.reshape(HQ, HD, D)[:, _ROPE_PERM, :].reshape(D, D)
    wk_p = wk.reshape(HKV, HD, D)[:, _ROPE_PERM, :].reshape(HKV * HD, D)
    wqT = np.ascontiguousarray(wq_p.T).astype(bf16)   # [D, D]
    wkT = np.ascontiguousarray(wk_p.T).astype(bf16)   # [D, 512]
    wvT = np.ascontiguousarray(wv.T).astype(bf16)     # [D, 512]
    woT = np.ascontiguousarray(wo.T).astype(bf16)     # [D, D]

    in_maps = []
    for c in range(NCORES):
        b, g = divmod(c, 2)
        wkv = np.concatenate([wkT[:, g * KVO:(g + 1) * KVO],
                              wvT[:, g * KVO:(g + 1) * KVO]], axis=1)
        in_maps.append({
            "xT": np.ascontiguousarray(x[b].T).astype(bf16),
            "wqT": np.ascontiguousarray(wqT[:, g * GO:(g + 1) * GO]),
            "wkvT": np.ascontiguousarray(wkv),
            "woT": np.ascontiguousarray(woT[g * GO:(g + 1) * GO, :]),
            "Ct": Ct,
            "St": St,
        })
    return in_maps


def kernel(x, wq, wk, wv, wo, temporal_pos, structural_pos, _trace=False):
    nc = build_nc(with_collective=True)
    in_maps = make_in_maps(x, wq, wk, wv, wo, temporal_pos, structural_pos)
    res = bass_utils.run_bass_kernel_spmd(
        nc, in_maps, core_ids=list(range(NCORES)), trace=_trace)
    out = np.stack([
        np.concatenate([res.results[2 * b]["y"], res.results[2 * b + 1]["y"]],
                       axis=0) for b in range(B)])
    kernel.last_result = res
    return out.astype(np.float32)


# revision 16
# speedup vs baseline: 1.0205x; 1.0178x over previous
"""GQA attention with 2D RoPE on 8 TRN2 NeuronCores — v2.

Sharding: batch data-parallel x4  X  head-group tensor-parallel x2.
Core c handles batch b=c//2 and head group g=c%2 (16 Q heads, 4 KV heads).
wo is row-sharded; partials are ReduceScattered per 512-col block across
each core pair, so core 2b returns rows 0:448 and core 2b+1 rows 448:896.

v2 vs v1 (same math):
  - merged DMAs (multi-dim APs): x in 4, wkv in 2, wq in 2, wo in 1,
    rec-bounces batched per head, cc_in / y path per-oc merges.
  - engine rebalance: masks on DVE, AT-muls on gpsimd(Pool), V evicts on
    Pool, K/Q psum evicts on DVE, phase-3 psum evicts + y-cast on Act.
  - issue-order software pipelining: per head scores groups lead AV groups
    by ~2 units to hide exp latency; Q og=1 projection quarters zipped
    between attention head pairs to keep PE fed while Act drains exps.

PSUM (8 banks): ph1a: psv 4 (2 b_ packed/bank) + psk 4.  ph1b: psq 8.
ph2: pss 2 tags x [128,4,256] (2 banks each) + psav 2 x [128,224] (1 bank)
+ psq1 2 x [128,448] (1 bank).  ph3: pso 7 x [128,512].
"""

import math
import numpy as np

import concourse.bass as bass
import concourse.tile as tile
import concourse.mybir as mybir
from concourse import bacc
from concourse import bass_utils

F32 = mybir.dt.float32
BF16 = mybir.dt.bfloat16
I32 = mybir.dt.int32
AF = mybir.ActivationFunctionType
ALU = mybir.AluOpType

B, L, D = 4, 896, 2048
HQ, HKV, HD = 32, 8, 64
NCORES = 8
GO = D // 2          # 1024 q-out dims per core
KVO = HKV * HD // 2  # 256 kv-out dims per core
NH = 16              # q heads per core
NKV = 4              # kv heads per core
P = 128
NI = D // P          # 16 contraction chunks
LB = L // P          # 7 key/l blocks
QCN = 4              # q chunks
QCW = L // QCN       # 224 q-chunk width
NKB = [2, 4, 6, 7]   # key blocks per q chunk (causal)
ROPE_BASE = 10000.0
TWO_PI = 2.0 * math.pi

# (qc, kb) pairs needing a causal mask, with affine_select base = 224*qc - 128*kb
PARTIAL = {}
for _qc in range(QCN):
    for _kb in range(NKB[_qc]):
        lo_key, hi_key = 128 * _kb, 128 * _kb + 127
        lo_row, hi_row = QCW * _qc, QCW * (_qc + 1) - 1
        if hi_key > lo_row:  # some key exceeds some row -> partial
            PARTIAL[(_qc, _kb)] = QCW * _qc - 128 * _kb

_NC_CACHE = {}


def build_nc(with_collective=True):
    key = with_collective
    if key in _NC_CACHE:
        return _NC_CACHE[key]
    nc = bacc.Bacc("TRN2", target_bir_lowering=False, debug=False,
                   num_devices=NCORES)
    ins = {
        "xT": nc.dram_tensor("xT", [D, L], BF16, kind="ExternalInput").ap(),
        "wqT": nc.dram_tensor("wqT", [D, GO], BF16, kind="ExternalInput").ap(),
        "wkvT": nc.dram_tensor("wkvT", [D, 2 * KVO], BF16,
                               kind="ExternalInput").ap(),
        "woT": nc.dram_tensor("woT", [GO, D], BF16, kind="ExternalInput").ap(),
        "Ct": nc.dram_tensor("Ct", [P, L], BF16, kind="ExternalInput").ap(),
        "St": nc.dram_tensor("St", [P, L], BF16, kind="ExternalInput").ap(),
    }
    y = nc.dram_tensor("y", [L // 2, D], F32, kind="ExternalOutput").ap()
    with tile.TileContext(nc) as tc:
        _build_kernel(nc, tc, ins, y, with_collective)
    nc.compile()
    _NC_CACHE[key] = nc
    return nc


def _ap3(dram_ap, row0, nrow_groups, group, ncols):
    """[128, nrow_groups, ncols] AP over dram rows row0.. in groups of 128.

    (p, c, f) -> row (row0 + 128*c + p), col f.
    """
    return bass.AP(tensor=dram_ap.tensor,
                   offset=dram_ap.offset + row0 * ncols,
                   ap=[[ncols, P], [group, nrow_groups], [1, ncols]])


def _bcast_row(dram_ap, parts, n):
    return bass.AP(tensor=dram_ap.tensor, offset=dram_ap.offset,
                   ap=[[0, parts], [1, n]])


def _rope(nc, pool, t, C, S):
    """t = t*C + shuffle16(t)*S, fully in place."""
    shuf = pool.tile([P, L], BF16, tag="rope_shuf")
    mask = [(p ^ 16) for p in range(32)]
    nc.vector.stream_shuffle(shuf[:], t[:], mask)
    nc.vector.tensor_mul(t[:], t[:], C[:])
    nc.vector.tensor_mul(shuf[:], shuf[:], S[:])
    nc.vector.tensor_add(t[:], t[:], shuf[:])


def _build_kernel(nc, tc, ins, y, with_collective):
    import contextlib
    ctx = contextlib.ExitStack()
    with ctx:
        const = ctx.enter_context(tc.tile_pool(name="const", bufs=1))

        # -------- persistent (whole-kernel) activation storage ----------
        big = ctx.enter_context(tc.tile_pool(name="big", bufs=1))
        WOB = big.tile([P, 8, D], BF16, tag="wob", name="wob")
        QT = [big.tile([P, L], BF16, tag=f"qt{i}", name=f"qt{i}")
              for i in range(8)]
        KTd = [big.tile([P, L], BF16, tag=f"kt{i}", name=f"kt{i}")
               for i in range(NKV)]
        # Vext[kv][kb][variant]: variant 0 = [V|1], 1 = [1|V]
        Vext = [[[big.tile([P, P], BF16, tag=f"v{k}_{b_}_{vr}",
                           name=f"v{k}_{b_}_{vr}")
                  for vr in range(2)] for b_ in range(LB)] for k in range(NKV)]
        AT = [big.tile([P, L], BF16, tag=f"at{i}", name=f"at{i}")
              for i in range(8)]

        # attention-phase pools (opened before proj so proj can pop first)
        ev = ctx.enter_context(tc.tile_pool(name="ev", bufs=2))
        upool = ctx.enter_context(tc.tile_pool(name="uatt", bufs=2))
        recpool = ctx.enter_context(tc.tile_pool(name="rec", bufs=2))
        # proj pool: closed after attention so phase 3 can reuse the space
        proj = tc.tile_pool(name="proj", bufs=1)
        prj = proj.__enter__()
        XT = prj.tile([P, NI, L], BF16, tag="xt", name="xt")
        WKV = prj.tile([P, NI, 2 * KVO], BF16, tag="wkv", name="wkv")
        WQ = prj.tile([P, NI, GO], BF16, tag="wq", name="wq")

        # ------------- rope tables (host-precomputed) + causal masks ----
        C = const.tile([P, L], BF16, tag="C", name="C")
        S = const.tile([P, L], BF16, tag="S", name="S")
        nc.sync.dma_start(C[:], ins["Ct"])
        nc.sync.dma_start(S[:], ins["St"])
        masks = {}
        for (qc, kb), base in PARTIAL.items():
            m = const.tile([P, QCW], BF16, tag=f"mask{qc}_{kb}",
                           name=f"mask{qc}_{kb}")
            nc.vector.memset(m[:], 1.0)
            nc.gpsimd.affine_select(out=m[:], in_=m[:], compare_op=ALU.is_ge,
                                    fill=0.0, base=base, channel_multiplier=-1,
                                    pattern=[[1, QCW]])
            masks[(qc, kb)] = m

        # ------------- input DMAs (SP queue, issue order = priority) ----
        nc.sync.dma_start(XT[:, 0:1, :], _ap3(ins["xT"], 0, 1, P * L, L))
        nc.sync.dma_start(WKV[:, 0:1, :],
                          _ap3(ins["wkvT"], 0, 1, P * 2 * KVO, 2 * KVO))
        nc.sync.dma_start(XT[:, 1:2, :], _ap3(ins["xT"], 128, 1, P * L, L))
        nc.sync.dma_start(WKV[:, 1:4, :],
                          _ap3(ins["wkvT"], 128, 3, P * 2 * KVO, 2 * KVO))
        for c8 in range(1, 8):  # rest of x in 2-chunk pieces, wkv interleaved
            nc.sync.dma_start(XT[:, 2 * c8:2 * c8 + 2, :],
                              _ap3(ins["xT"], 256 * c8, 2, P * L, L))
            if c8 < 4:
                nc.sync.dma_start(
                    WKV[:, 4 * c8:4 * c8 + 4, :],
                    _ap3(ins["wkvT"], 512 * c8, 4, P * 2 * KVO, 2 * KVO))
        for c2 in range(2):   # wq in 2 DMAs of 8 chunks
            nc.sync.dma_start(WQ[:, 8 * c2:8 * c2 + 8, :],
                              _ap3(ins["wqT"], 1024 * c2, 8, P * GO, GO))
        # ones halves of Vext (Pool; idle at start)
        for k in range(NKV):
            for b_ in range(LB):
                nc.gpsimd.memset(Vext[k][b_][0][:, 64:128], 1.0)
                nc.gpsimd.memset(Vext[k][b_][1][:, 0:64], 1.0)

        def qcopy(psq_a, psq_b, qt):
            nc.scalar.copy(qt[0:P, 0:448], psq_a[:])
            nc.scalar.copy(qt[0:P, 448:896], psq_b[:])

        def og0_half(psq_pool, half, tagbase):
            psq = [psq_pool.tile([P, 448], F32, tag=f"{tagbase}{j}",
                                 name=f"q0_{half}_{j}") for j in range(4)]
            for i in range(NI):
                for obh in range(2):
                    ob = half * 2 + obh
                    for h2 in range(2):
                        nc.tensor.matmul(
                            psq[obh * 2 + h2][:],
                            WQ[:, i, ob * P:(ob + 1) * P],
                            XT[:, i, h2 * 448:(h2 + 1) * 448],
                            start=(i == 0), stop=(i == NI - 1))
            return psq

        # ---------------- phase 1: V -> K -> Q og0, staged bank reuse ---
        with tc.tile_pool(name="ps1", bufs=1, space="PSUM") as ps1:
            psv = [ps1.tile([P, KVO], F32, tag=f"p{j}", name=f"pv{j}")
                   for j in range(LB)]
            for i in range(NI):
                st, sp = (i == 0), (i == NI - 1)
                for b_ in range(LB):
                    nc.tensor.matmul(
                        psv[b_][:],
                        XT[:, i, b_ * P:(b_ + 1) * P],
                        WKV[:, i, KVO:2 * KVO], start=st, stop=sp)
            # V evict (Act): psum -> Vext variants; b_ 0-3 first (K reuses)
            for b_ in range(LB):
                for k in range(NKV):
                    sl = psv[b_][:, k * 64:(k + 1) * 64]
                    nc.scalar.copy(Vext[k][b_][0][:, 0:64], sl)
                    nc.vector.tensor_copy(Vext[k][b_][1][:, 64:128], sl)

            psk = [ps1.tile([P, 448], F32, tag=f"p{j}", name=f"pk{j}")
                   for j in range(4)]
            for i in range(NI):
                st, sp = (i == 0), (i == NI - 1)
                for ob in range(2):
                    for h2 in range(2):
                        nc.tensor.matmul(
                            psk[ob * 2 + h2][:],
                            WKV[:, i, ob * P:(ob + 1) * P],
                            XT[:, i, h2 * 448:(h2 + 1) * 448],
                            start=st, stop=sp)
            # K evict (Act) + rope (DVE) + duplicate into KTd halves
            for ob in range(2):
                roped = ev.tile([P, L], BF16, tag="roped")
                for h2 in range(2):
                    nc.scalar.copy(roped[:, h2 * 448:(h2 + 1) * 448],
                                   psk[ob * 2 + h2][:])
                _rope(nc, ev, roped, C, S)
                for sub in range(2):
                    k = ob * 2 + sub
                    src = roped[sub * 64:(sub + 1) * 64, :]
                    nc.sync.dma_start(KTd[k][0:64, :], src)
                    nc.sync.dma_start(KTd[k][64:128, :], src)

            def og0_half(tags, half):
                psq = [ps1.tile([P, 448], F32, tag=f"p{t}",
                                name=f"q0_{half}_{j}")
                       for j, t in enumerate(tags)]
                for i in range(NI):
                    for obh in range(2):
                        ob = half * 2 + obh
                        for h2 in range(2):
                            nc.tensor.matmul(
                                psq[obh * 2 + h2][:],
                                WQ[:, i, ob * P:(ob + 1) * P],
                                XT[:, i, h2 * 448:(h2 + 1) * 448],
                                start=(i == 0), stop=(i == NI - 1))
                return psq

            def qcopy(psq_a, psq_b, qt):
                nc.scalar.copy(qt[0:P, 0:448], psq_a[:])
                nc.scalar.copy(qt[0:P, 448:896], psq_b[:])

            # og0a on banks 4-6 + a fresh one (free after V evicts b_ 4-6)
            psq = og0_half((4, 5, 6, 7), 0)
            qcopy(psq[0], psq[1], QT[0])
            qcopy(psq[2], psq[3], QT[1])
            _rope(nc, ev, QT[0], C, S)     # runs on DVE during og0b
            _rope(nc, ev, QT[1], C, S)
            psqb = og0_half((0, 1, 2, 3), 1)   # K banks (evicted during og0a)
            qcopy(psqb[0], psqb[1], QT[2])
            qcopy(psqb[2], psqb[3], QT[3])
            _rope(nc, ev, QT[2], C, S)
            _rope(nc, ev, QT[3], C, S)

        # ---------------- phase 2 + zipped Q og=1 -----------------------
        pss_cm = tc.tile_pool(name="pss", bufs=1, space="PSUM")
        pss = pss_cm.__enter__()
        psav_cm = tc.tile_pool(name="psav", bufs=1, space="PSUM")
        psav = psav_cm.__enter__()
        s_ctr = [0]   # global score-slot rotation (shared with og1 eighths)

        def s_tile(name):
            t = pss.tile([P, 4, 256], F32, tag=f"s{s_ctr[0] % 2}", name=name)
            s_ctr[0] += 1
            return t

        def og1_eighth(j):
            """One (ob, h2) og=1 accumulation -> immediate Pool evict."""
            ob, h2 = j // 2, j % 2
            pq = pss.tile([P, 448], F32, tag=f"s{s_ctr[0] % 2}",
                          name=f"q1_{j}")
            s_ctr[0] += 1
            for i in range(NI):
                nc.tensor.matmul(
                    pq[:],
                    WQ[:, i, 512 + ob * P:512 + (ob + 1) * P],
                    XT[:, i, h2 * 448:(h2 + 1) * 448],
                    start=(i == 0), stop=(i == NI - 1))
            nc.vector.tensor_copy(QT[4 + ob][:, h2 * 448:(h2 + 1) * 448],
                                  pq[:])
            if h2 == 1:
                _rope(nc, ev, QT[4 + ob], C, S)

        def attention_head(h):
            kv = h // 4
            qblk, qsub = divmod(h, 2)
            qoff = qsub * 64
            soff = 64 - qoff
            vr = qsub
            # flat score/exp groups: (qc, k0, ng, U)
            groups = []
            for qc in range(QCN):
                nkb = NKB[qc]
                for k0 in range(0, nkb, 4):
                    groups.append((qc, k0, min(4, nkb - k0)))
            # AV psum: qc pairs packed 2-per-bank so only 2 tags are live
            av_tiles = {}
            recs = recpool.tile([P, QCN, QCW], F32, tag="recs", bufs=1,
                                name=f"recs{h}")
            done_u = {}

            def emit_scores(gi):
                qc, k0, ng = groups[gi]
                qsl = slice(qc * QCW, (qc + 1) * QCW)
                ps_s = s_tile(f"s{h}_{qc}_{k0}")
                for j in range(ng):
                    nc.tensor.matmul(
                        ps_s[:, j, 0:QCW],
                        KTd[kv][qoff:qoff + 64,
                                (k0 + j) * P:(k0 + j + 1) * P],
                        QT[qblk][qoff:qoff + 64, qsl],
                        start=True, stop=True, tile_position=(qoff, 0))
                U = upool.tile([P, 4, QCW], BF16, tag=f"u{gi % 2}",
                               name=f"u{h}_{qc}_{k0}")
                nc.scalar.activation(U[:, 0:ng, 0:QCW], ps_s[:, 0:ng, 0:QCW],
                                     AF.Exp, scale=0.125)
                for j in range(ng):
                    if (qc, k0 + j) in PARTIAL:
                        eng = nc.gpsimd if (h + j) % 2 else nc.vector
                        eng.tensor_tensor(
                            U[:, j, 0:QCW], U[:, j, 0:QCW],
                            masks[(qc, k0 + j)][:], op=ALU.mult)
                done_u[(qc, k0)] = (ng, U)

            def emit_av(qc):
                nkb = NKB[qc]
                if qc % 2 == 0:
                    av_tiles[qc // 2] = psav.tile(
                        [P, 2, QCW], F32, tag=f"av{qc // 2}", bufs=2,
                        name=f"av{h}_{qc // 2}")
                ps_av = av_tiles[qc // 2][:, qc % 2, :]
                kb = 0
                for k0 in range(0, nkb, 4):
                    ng, U = done_u[(qc, k0)]
                    for j in range(ng):
                        nc.tensor.matmul(
                            ps_av, Vext[kv][k0 + j][vr][:],
                            U[:, j, 0:QCW],
                            start=(kb == 0), stop=(kb == nkb - 1))
                        kb += 1
                if qc % 2 == 1:   # one reciprocal per qc pair
                    nc.vector.reciprocal(
                        recs[soff:soff + 64, qc - 1:qc + 1, :],
                        av_tiles[qc // 2][soff:soff + 64, :, :])

            # interleave: scores lead AV by ~2 groups
            order = [("s", 0), ("s", 1), ("s", 2), ("av", 0), ("s", 3),
                     ("av", 1), ("s", 4), ("av", 2), ("s", 5), ("av", 3)]
            for kind, idx in order:
                if kind == "s":
                    emit_scores(idx)
                else:
                    emit_av(idx)

            # one partition-shift DMA for all 4 reciprocals of this head
            rec = recpool.tile([P, QCN, QCW], F32, tag="rec", bufs=1,
                                name=f"rec{h}")
            nc.sync.dma_start(rec[qoff:qoff + 64, :, :],
                              recs[soff:soff + 64, :, :])
            # AT writes (DVE), one op per qc pair
            for c in range(2):
                qsl = slice(2 * c * QCW, 2 * (c + 1) * QCW)
                nc.vector.tensor_tensor(
                    AT[qblk][qoff:qoff + 64, qsl],
                    av_tiles[c][qoff:qoff + 64, :, :],
                    rec[qoff:qoff + 64, 2 * c:2 * c + 2, :], op=ALU.mult)

        for h in range(8):
            attention_head(h)
            og1_eighth(h)
            if h in (2, 3, 4, 5):   # wo load mid-attention, 4 pieces
                c = h - 2
                nc.sync.dma_start(WOB[:, 2 * c:2 * c + 2, :],
                                  _ap3(ins["woT"], 256 * c, 2, P * D, D))
        for h in range(8, 16):
            attention_head(h)
        psav_cm.__exit__(None, None, None)
        pss_cm.__exit__(None, None, None)
        proj.__exit__(None, None, None)

        # ---------------- phase 3: out projection + reduce-scatter -------
        # 2 collectives (15us fixed cost each in the model): one per oc pair
        with tc.tile_pool(name="osb", bufs=2) as osb, \
             tc.tile_pool(name="pso", bufs=1, space="PSUM") as pso, \
             tc.tile_pool(name="ccdram", bufs=1, space="DRAM") as ccdram:
            cc_in = [ccdram.tile([L, 1024], BF16, tag=f"ccin{g_}",
                                 name=f"ccin{g_}") for g_ in range(2)]
            cc_out = [ccdram.tile([L // 2, 1024], BF16, tag=f"ccout{g_}",
                                  name=f"ccout{g_}") for g_ in range(2)]
            for oc in range(4):
                g_, half = divmod(oc, 2)
                pso_t = [pso.tile([P, 512], F32, tag=f"po{b_}",
                                  name=f"pso{oc}_{b_}") for b_ in range(LB)]
                ot = osb.tile([P, LB, 512], BF16, tag="ot", name=f"ot{oc}")
                for b_ in range(LB):   # b_-outer: evict+send as soon as done
                    for ic in range(8):
                        nc.tensor.matmul(
                            pso_t[b_][:], AT[ic][:, b_ * P:(b_ + 1) * P],
                            WOB[:, ic, oc * 512:(oc + 1) * 512],
                            start=(ic == 0), stop=(ic == 7))
                    nc.scalar.copy(ot[:, b_, :], pso_t[b_][:])
                    nc.sync.dma_start(
                        bass.AP(tensor=cc_in[g_].tensor,
                                offset=(cc_in[g_].offset + b_ * P * 1024
                                        + half * 512),
                                ap=[[1024, P], [1, 512]]),
                        ot[:, b_, :])
                if half == 1 and with_collective:
                    # collective issued now (Pool queue, fires on input-ready);
                    # y-paths deferred so later oc evict DMAs aren't stuck
                    # behind RS-gated reads in the SP queue.
                    nc.gpsimd.collective_compute(
                        "ReduceScatter", ALU.add,
                        replica_groups=[[0, 1], [2, 3], [4, 5], [6, 7]],
                        ins=[cc_in[g_].opt()], outs=[cc_out[g_].opt()])
            for g_ in range(2):
                src_dram = cc_out[g_] if with_collective else cc_in[g_]
                # bf16 -> f32 via SBUF bounce, 128-row pipelined chunks
                for r0 in (0, 128, 256, 384):
                    rn = 64 if r0 == 384 else P
                    yb = osb.tile([P, 1024], BF16, tag="yb",
                                  name=f"yb{g_}_{r0}")
                    nc.sync.dma_start(
                        yb[0:rn, :],
                        bass.AP(tensor=src_dram.tensor,
                                offset=src_dram.offset + r0 * 1024,
                                ap=[[1024, rn], [1, 1024]]))
                    yf = osb.tile([P, 1024], F32, tag="yf",
                                  name=f"yf{g_}_{r0}")
                    nc.scalar.copy(yf[0:rn, :], yb[0:rn, :])
                    nc.sync.dma_start(
                        bass.AP(tensor=y.tensor,
                                offset=(y.offset + r0 * D + g_ * 1024),
                                ap=[[D, rn], [1, 1024]]),
                        yf[0:rn, :])


# ---------------------------------------------------------------- host side
_ROPE_PERM = np.concatenate([
    np.arange(0, 32, 2), np.arange(1, 32, 2),
    np.arange(32, 64, 2), np.arange(33, 64, 2)])


def make_in_maps(x, wq, wk, wv, wo, temporal_pos, structural_pos):
    import ml_dtypes
    bf16 = ml_dtypes.bfloat16
    x = np.asarray(x, dtype=np.float32)
    wq = np.asarray(wq, dtype=np.float32)
    wk = np.asarray(wk, dtype=np.float32)
    wv = np.asarray(wv, dtype=np.float32)
    wo = np.asarray(wo, dtype=np.float32)
    pt = np.asarray(temporal_pos).astype(np.float64)
    ps = np.asarray(structural_pos).astype(np.float64)
    inv = 1.0 / (10000.0 ** (np.arange(16) / 16.0))
    ct, st = np.cos(pt[:, None] * inv).T, np.sin(pt[:, None] * inv).T
    cs, ss = np.cos(ps[:, None] * inv).T, np.sin(ps[:, None] * inv).T
    Ct = np.concatenate([ct, ct, cs, cs] * 2).astype(bf16)     # [128, 896]
    St = np.concatenate([-st, st, -ss, ss] * 2).astype(bf16)

    wq_p = wq.reshape(HQ, HD, D)[:, _ROPE_PERM, :].reshape(D, D)
    wk_p = wk.reshape(HKV, HD, D)[:, _ROPE_PERM, :].reshape(HKV * HD, D)
    wqT = np.ascontiguousarray(wq_p.T).astype(bf16)   # [D, D]
    wkT = np.ascontiguousarray(wk_p.T).astype(bf16)   # [D, 512]
    wvT = np.ascontiguousarray(wv.T).astype(bf16)     # [D, 512]
    woT = np.ascontiguousarray(wo.T).astype(bf16)     # [D, D]

    in_maps = []
    for c in range(NCORES):
        b, g = divmod(c, 2)
        wkv = np.concatenate([wkT[:, g * KVO:(g + 1) * KVO],
                              wvT[:, g * KVO:(g + 1) * KVO]], axis=1)
        in_maps.append({
            "xT": np.ascontiguousarray(x[b].T).astype(bf16),
            "wqT": np.ascontiguousarray(wqT[:, g * GO:(g + 1) * GO]),
            "wkvT": np.ascontiguousarray(wkv),
            "woT": np.ascontiguousarray(woT[g * GO:(g + 1) * GO, :]),
            "Ct": Ct,
            "St": St,
        })
    return in_maps


def kernel(x, wq, wk, wv, wo, temporal_pos, structural_pos, _trace=False):
    nc = build_nc(with_collective=True)
    in_maps = make_in_maps(x, wq, wk, wv, wo, temporal_pos, structural_pos)
    res = bass_utils.run_bass_kernel_spmd(
        nc, in_maps, core_ids=list(range(NCORES)), trace=_trace)
    out = np.stack([
        np.concatenate([res.results[2 * b]["y"], res.results[2 * b + 1]["y"]],
                       axis=0) for b in range(B)])
    kernel.last_result = res
    return out.astype(np.float32)


# revision 17
# speedup vs baseline: 1.0251x; 1.0045x over previous
"""GQA attention with 2D RoPE on 8 TRN2 NeuronCores — v2.

Sharding: batch data-parallel x4  X  head-group tensor-parallel x2.
Core c handles batch b=c//2 and head group g=c%2 (16 Q heads, 4 KV heads).
wo is row-sharded; partials are ReduceScattered per 512-col block across
each core pair, so core 2b returns rows 0:448 and core 2b+1 rows 448:896.

v2 vs v1 (same math):
  - merged DMAs (multi-dim APs): x in 4, wkv in 2, wq in 2, wo in 1,
    rec-bounces batched per head, cc_in / y path per-oc merges.
  - engine rebalance: masks on DVE, AT-muls on gpsimd(Pool), V evicts on
    Pool, K/Q psum evicts on DVE, phase-3 psum evicts + y-cast on Act.
  - issue-order software pipelining: per head scores groups lead AV groups
    by ~2 units to hide exp latency; Q og=1 projection quarters zipped
    between attention head pairs to keep PE fed while Act drains exps.

PSUM (8 banks): ph1a: psv 4 (2 b_ packed/bank) + psk 4.  ph1b: psq 8.
ph2: pss 2 tags x [128,4,256] (2 banks each) + psav 2 x [128,224] (1 bank)
+ psq1 2 x [128,448] (1 bank).  ph3: pso 7 x [128,512].
"""

import math
import numpy as np

import concourse.bass as bass
import concourse.tile as tile
import concourse.mybir as mybir
from concourse import bacc
from concourse import bass_utils

F32 = mybir.dt.float32
BF16 = mybir.dt.bfloat16
I32 = mybir.dt.int32
AF = mybir.ActivationFunctionType
ALU = mybir.AluOpType

B, L, D = 4, 896, 2048
HQ, HKV, HD = 32, 8, 64
NCORES = 8
GO = D // 2          # 1024 q-out dims per core
KVO = HKV * HD // 2  # 256 kv-out dims per core
NH = 16              # q heads per core
NKV = 4              # kv heads per core
P = 128
NI = D // P          # 16 contraction chunks
LB = L // P          # 7 key/l blocks
QCN = 4              # q chunks
QCW = L // QCN       # 224 q-chunk width
NKB = [2, 4, 6, 7]   # key blocks per q chunk (causal)
ROPE_BASE = 10000.0
TWO_PI = 2.0 * math.pi

# (qc, kb) pairs needing a causal mask, with affine_select base = 224*qc - 128*kb
PARTIAL = {}
for _qc in range(QCN):
    for _kb in range(NKB[_qc]):
        lo_key, hi_key = 128 * _kb, 128 * _kb + 127
        lo_row, hi_row = QCW * _qc, QCW * (_qc + 1) - 1
        if hi_key > lo_row:  # some key exceeds some row -> partial
            PARTIAL[(_qc, _kb)] = QCW * _qc - 128 * _kb

_NC_CACHE = {}


def build_nc(with_collective=True):
    key = with_collective
    if key in _NC_CACHE:
        return _NC_CACHE[key]
    nc = bacc.Bacc("TRN2", target_bir_lowering=False, debug=False,
                   num_devices=NCORES)
    ins = {
        "xT": nc.dram_tensor("xT", [D, L], BF16, kind="ExternalInput").ap(),
        "wqT": nc.dram_tensor("wqT", [D, GO], BF16, kind="ExternalInput").ap(),
        "wkvT": nc.dram_tensor("wkvT", [D, 2 * KVO], BF16,
                               kind="ExternalInput").ap(),
        "woT": nc.dram_tensor("woT", [GO, D], BF16, kind="ExternalInput").ap(),
        "Ct": nc.dram_tensor("Ct", [P, L], BF16, kind="ExternalInput").ap(),
        "St": nc.dram_tensor("St", [P, L], BF16, kind="ExternalInput").ap(),
    }
    y = nc.dram_tensor("y", [L // 2, D], F32, kind="ExternalOutput").ap()
    with tile.TileContext(nc) as tc:
        _build_kernel(nc, tc, ins, y, with_collective)
    nc.compile()
    _NC_CACHE[key] = nc
    return nc


def _ap3(dram_ap, row0, nrow_groups, group, ncols):
    """[128, nrow_groups, ncols] AP over dram rows row0.. in groups of 128.

    (p, c, f) -> row (row0 + 128*c + p), col f.
    """
    return bass.AP(tensor=dram_ap.tensor,
                   offset=dram_ap.offset + row0 * ncols,
                   ap=[[ncols, P], [group, nrow_groups], [1, ncols]])


def _bcast_row(dram_ap, parts, n):
    return bass.AP(tensor=dram_ap.tensor, offset=dram_ap.offset,
                   ap=[[0, parts], [1, n]])


def _rope(nc, pool, t, C, S):
    """t = t*C + shuffle16(t)*S, fully in place."""
    shuf = pool.tile([P, L], BF16, tag="rope_shuf")
    mask = [(p ^ 16) for p in range(32)]
    nc.vector.stream_shuffle(shuf[:], t[:], mask)
    nc.vector.tensor_mul(t[:], t[:], C[:])
    nc.vector.tensor_mul(shuf[:], shuf[:], S[:])
    nc.vector.tensor_add(t[:], t[:], shuf[:])


def _build_kernel(nc, tc, ins, y, with_collective):
    import contextlib
    ctx = contextlib.ExitStack()
    with ctx:
        const = ctx.enter_context(tc.tile_pool(name="const", bufs=1))

        # -------- persistent (whole-kernel) activation storage ----------
        big = ctx.enter_context(tc.tile_pool(name="big", bufs=1))
        WOB = big.tile([P, 8, D], BF16, tag="wob", name="wob")
        QT = [big.tile([P, L], BF16, tag=f"qt{i}", name=f"qt{i}")
              for i in range(8)]
        KTd = [big.tile([P, L], BF16, tag=f"kt{i}", name=f"kt{i}")
               for i in range(NKV)]
        # Vext[kv][kb][variant]: variant 0 = [V|1], 1 = [1|V]
        Vext = [[[big.tile([P, P], BF16, tag=f"v{k}_{b_}_{vr}",
                           name=f"v{k}_{b_}_{vr}")
                  for vr in range(2)] for b_ in range(LB)] for k in range(NKV)]
        AT = [big.tile([P, L], BF16, tag=f"at{i}", name=f"at{i}")
              for i in range(8)]

        # attention-phase pools (opened before proj so proj can pop first)
        ev = ctx.enter_context(tc.tile_pool(name="ev", bufs=2))
        upool = ctx.enter_context(tc.tile_pool(name="uatt", bufs=2))
        recpool = ctx.enter_context(tc.tile_pool(name="rec", bufs=2))
        # proj pool: closed after attention so phase 3 can reuse the space
        proj = tc.tile_pool(name="proj", bufs=1)
        prj = proj.__enter__()
        XT = prj.tile([P, NI, L], BF16, tag="xt", name="xt")
        WKV = prj.tile([P, NI, 2 * KVO], BF16, tag="wkv", name="wkv")
        WQ = prj.tile([P, NI, GO], BF16, tag="wq", name="wq")

        # ------------- rope tables (host-precomputed) + causal masks ----
        C = const.tile([P, L], BF16, tag="C", name="C")
        S = const.tile([P, L], BF16, tag="S", name="S")
        nc.sync.dma_start(C[:], ins["Ct"])
        nc.sync.dma_start(S[:], ins["St"])
        masks = {}
        for (qc, kb), base in PARTIAL.items():
            m = const.tile([P, QCW], BF16, tag=f"mask{qc}_{kb}",
                           name=f"mask{qc}_{kb}")
            nc.vector.memset(m[:], 1.0)
            nc.gpsimd.affine_select(out=m[:], in_=m[:], compare_op=ALU.is_ge,
                                    fill=0.0, base=base, channel_multiplier=-1,
                                    pattern=[[1, QCW]])
            masks[(qc, kb)] = m

        # ------------- input DMAs (SP queue, issue order = priority) ----
        nc.sync.dma_start(XT[:, 0:1, :], _ap3(ins["xT"], 0, 1, P * L, L))
        nc.sync.dma_start(WKV[:, 0:1, :],
                          _ap3(ins["wkvT"], 0, 1, P * 2 * KVO, 2 * KVO))
        nc.sync.dma_start(XT[:, 1:2, :], _ap3(ins["xT"], 128, 1, P * L, L))
        nc.sync.dma_start(WKV[:, 1:4, :],
                          _ap3(ins["wkvT"], 128, 3, P * 2 * KVO, 2 * KVO))
        for c8 in range(1, 8):  # rest of x in 2-chunk pieces, wkv interleaved
            nc.sync.dma_start(XT[:, 2 * c8:2 * c8 + 2, :],
                              _ap3(ins["xT"], 256 * c8, 2, P * L, L))
            if c8 < 4:
                nc.sync.dma_start(
                    WKV[:, 4 * c8:4 * c8 + 4, :],
                    _ap3(ins["wkvT"], 512 * c8, 4, P * 2 * KVO, 2 * KVO))
        for c2 in range(2):   # wq in 2 DMAs of 8 chunks
            nc.sync.dma_start(WQ[:, 8 * c2:8 * c2 + 8, :],
                              _ap3(ins["wqT"], 1024 * c2, 8, P * GO, GO))
        # ones halves of Vext (Pool; idle at start)
        for k in range(NKV):
            for b_ in range(LB):
                nc.gpsimd.memset(Vext[k][b_][0][:, 64:128], 1.0)
                nc.gpsimd.memset(Vext[k][b_][1][:, 0:64], 1.0)

        def qcopy(psq_a, psq_b, qt):
            nc.scalar.copy(qt[0:P, 0:448], psq_a[:])
            nc.scalar.copy(qt[0:P, 448:896], psq_b[:])

        def og0_half(psq_pool, half, tagbase):
            psq = [psq_pool.tile([P, 448], F32, tag=f"{tagbase}{j}",
                                 name=f"q0_{half}_{j}") for j in range(4)]
            for i in range(NI):
                for obh in range(2):
                    ob = half * 2 + obh
                    for h2 in range(2):
                        nc.tensor.matmul(
                            psq[obh * 2 + h2][:],
                            WQ[:, i, ob * P:(ob + 1) * P],
                            XT[:, i, h2 * 448:(h2 + 1) * 448],
                            start=(i == 0), stop=(i == NI - 1))
            return psq

        # ---------------- phase 1: V -> K -> Q og0, staged bank reuse ---
        with tc.tile_pool(name="ps1", bufs=1, space="PSUM") as ps1:
            psv = [ps1.tile([P, KVO], F32, tag=f"p{j}", name=f"pv{j}")
                   for j in range(LB)]
            for i in range(NI):
                st, sp = (i == 0), (i == NI - 1)
                for b_ in range(LB):
                    nc.tensor.matmul(
                        psv[b_][:],
                        XT[:, i, b_ * P:(b_ + 1) * P],
                        WKV[:, i, KVO:2 * KVO], start=st, stop=sp)
            # V evict (Act): psum -> Vext variants; b_ 0-3 first (K reuses)
            for b_ in range(LB):
                for k in range(NKV):
                    sl = psv[b_][:, k * 64:(k + 1) * 64]
                    nc.scalar.copy(Vext[k][b_][0][:, 0:64], sl)
                    nc.vector.tensor_copy(Vext[k][b_][1][:, 64:128], sl)

            psk = [ps1.tile([P, 448], F32, tag=f"p{j}", name=f"pk{j}")
                   for j in range(4)]
            for i in range(NI):
                st, sp = (i == 0), (i == NI - 1)
                for ob in range(2):
                    for h2 in range(2):
                        nc.tensor.matmul(
                            psk[ob * 2 + h2][:],
                            WKV[:, i, ob * P:(ob + 1) * P],
                            XT[:, i, h2 * 448:(h2 + 1) * 448],
                            start=st, stop=sp)
            # K evict (Act) + rope (DVE) + duplicate into KTd halves
            for ob in range(2):
                roped = ev.tile([P, L], BF16, tag="roped")
                for h2 in range(2):
                    nc.scalar.copy(roped[:, h2 * 448:(h2 + 1) * 448],
                                   psk[ob * 2 + h2][:])
                _rope(nc, ev, roped, C, S)
                for sub in range(2):
                    k = ob * 2 + sub
                    src = roped[sub * 64:(sub + 1) * 64, :]
                    nc.sync.dma_start(KTd[k][0:64, :], src)
                    nc.sync.dma_start(KTd[k][64:128, :], src)

            def og0_half(tags, half):
                psq = [ps1.tile([P, 448], F32, tag=f"p{t}",
                                name=f"q0_{half}_{j}")
                       for j, t in enumerate(tags)]
                for i in range(NI):
                    for obh in range(2):
                        ob = half * 2 + obh
                        for h2 in range(2):
                            nc.tensor.matmul(
                                psq[obh * 2 + h2][:],
                                WQ[:, i, ob * P:(ob + 1) * P],
                                XT[:, i, h2 * 448:(h2 + 1) * 448],
                                start=(i == 0), stop=(i == NI - 1))
                return psq

            def qcopy(psq_a, psq_b, qt):
                nc.scalar.copy(qt[0:P, 0:448], psq_a[:])
                nc.scalar.copy(qt[0:P, 448:896], psq_b[:])

            # og0a on banks 4-6 + a fresh one (free after V evicts b_ 4-6)
            psq = og0_half((4, 5, 6, 7), 0)
            qcopy(psq[0], psq[1], QT[0])
            qcopy(psq[2], psq[3], QT[1])
            _rope(nc, ev, QT[0], C, S)     # runs on DVE during og0b
            _rope(nc, ev, QT[1], C, S)
            psqb = og0_half((0, 1, 2, 3), 1)   # K banks (evicted during og0a)
            qcopy(psqb[0], psqb[1], QT[2])
            qcopy(psqb[2], psqb[3], QT[3])
            _rope(nc, ev, QT[2], C, S)
            _rope(nc, ev, QT[3], C, S)

        # ---------------- phase 2 + zipped Q og=1 -----------------------
        pss_cm = tc.tile_pool(name="pss", bufs=1, space="PSUM")
        pss = pss_cm.__enter__()
        psav_cm = tc.tile_pool(name="psav", bufs=1, space="PSUM")
        psav = psav_cm.__enter__()
        s_ctr = [0]   # global score-slot rotation (shared with og1 eighths)

        def s_tile(name):
            t = pss.tile([P, 4, 256], F32, tag=f"s{s_ctr[0] % 2}", name=name)
            s_ctr[0] += 1
            return t

        def og1_eighth(j):
            """One (ob, h2) og=1 accumulation -> immediate Pool evict."""
            ob, h2 = j // 2, j % 2
            pq = pss.tile([P, 448], F32, tag=f"s{s_ctr[0] % 2}",
                          name=f"q1_{j}")
            s_ctr[0] += 1
            for i in range(NI):
                nc.tensor.matmul(
                    pq[:],
                    WQ[:, i, 512 + ob * P:512 + (ob + 1) * P],
                    XT[:, i, h2 * 448:(h2 + 1) * 448],
                    start=(i == 0), stop=(i == NI - 1))
            nc.vector.tensor_copy(QT[4 + ob][:, h2 * 448:(h2 + 1) * 448],
                                  pq[:])
            if h2 == 1:
                _rope(nc, ev, QT[4 + ob], C, S)

        def attention_head(h):
            kv = h // 4
            qblk, qsub = divmod(h, 2)
            qoff = qsub * 64
            soff = 64 - qoff
            vr = qsub
            # flat score/exp groups: (qc, k0, ng, U)
            groups = []
            for qc in range(QCN):
                nkb = NKB[qc]
                for k0 in range(0, nkb, 4):
                    groups.append((qc, k0, min(4, nkb - k0)))
            # AV psum: qc pairs packed 2-per-bank so only 2 tags are live
            av_tiles = {}
            recs = recpool.tile([P, QCN, QCW], F32, tag="recs", bufs=1,
                                name=f"recs{h}")
            done_u = {}

            def emit_scores(gi):
                qc, k0, ng = groups[gi]
                qsl = slice(qc * QCW, (qc + 1) * QCW)
                ps_s = s_tile(f"s{h}_{qc}_{k0}")
                for j in range(ng):
                    nc.tensor.matmul(
                        ps_s[:, j, 0:QCW],
                        KTd[kv][qoff:qoff + 64,
                                (k0 + j) * P:(k0 + j + 1) * P],
                        QT[qblk][qoff:qoff + 64, qsl],
                        start=True, stop=True, tile_position=(qoff, 0))
                U = upool.tile([P, 4, QCW], BF16, tag=f"u{gi % 2}",
                               name=f"u{h}_{qc}_{k0}")
                nc.scalar.activation(U[:, 0:ng, 0:QCW], ps_s[:, 0:ng, 0:QCW],
                                     AF.Exp, scale=0.125)
                for j in range(ng):
                    if (qc, k0 + j) in PARTIAL:
                        eng = nc.gpsimd if (h + j) % 2 else nc.vector
                        eng.tensor_tensor(
                            U[:, j, 0:QCW], U[:, j, 0:QCW],
                            masks[(qc, k0 + j)][:], op=ALU.mult)
                done_u[(qc, k0)] = (ng, U)

            def emit_av(qc):
                nkb = NKB[qc]
                if qc % 2 == 0:
                    av_tiles[qc // 2] = psav.tile(
                        [P, 2, QCW], F32, tag=f"av{qc // 2}", bufs=2,
                        name=f"av{h}_{qc // 2}")
                ps_av = av_tiles[qc // 2][:, qc % 2, :]
                kb = 0
                for k0 in range(0, nkb, 4):
                    ng, U = done_u[(qc, k0)]
                    for j in range(ng):
                        nc.tensor.matmul(
                            ps_av, Vext[kv][k0 + j][vr][:],
                            U[:, j, 0:QCW],
                            start=(kb == 0), stop=(kb == nkb - 1))
                        kb += 1
                if qc % 2 == 1:   # one reciprocal per qc pair
                    nc.vector.reciprocal(
                        recs[soff:soff + 64, qc - 1:qc + 1, :],
                        av_tiles[qc // 2][soff:soff + 64, :, :])

            # interleave: scores lead AV by ~2 groups
            order = [("s", 0), ("s", 1), ("s", 2), ("av", 0), ("s", 3),
                     ("av", 1), ("s", 4), ("av", 2), ("s", 5), ("av", 3)]
            for kind, idx in order:
                if kind == "s":
                    emit_scores(idx)
                else:
                    emit_av(idx)

            # one partition-shift DMA for all 4 reciprocals of this head
            rec = recpool.tile([P, QCN, QCW], F32, tag="rec", bufs=1,
                                name=f"rec{h}")
            nc.sync.dma_start(rec[qoff:qoff + 64, :, :],
                              recs[soff:soff + 64, :, :])
            # AT writes (DVE), one op per qc pair
            for c in range(2):
                qsl = slice(2 * c * QCW, 2 * (c + 1) * QCW)
                nc.vector.tensor_tensor(
                    AT[qblk][qoff:qoff + 64, qsl],
                    av_tiles[c][qoff:qoff + 64, :, :],
                    rec[qoff:qoff + 64, 2 * c:2 * c + 2, :], op=ALU.mult)

        for h in range(8):
            attention_head(h)
            og1_eighth(h)
            if h in (2, 3, 4, 5):   # wo load mid-attention, 4 pieces
                c = h - 2
                nc.sync.dma_start(WOB[:, 2 * c:2 * c + 2, :],
                                  _ap3(ins["woT"], 256 * c, 2, P * D, D))
        for h in range(8, 16):
            attention_head(h)
        psav_cm.__exit__(None, None, None)
        pss_cm.__exit__(None, None, None)
        proj.__exit__(None, None, None)

        # ---------------- phase 3: out projection + reduce-scatter -------
        # 2 collectives (15us fixed cost each in the model): one per oc pair
        with tc.tile_pool(name="osb", bufs=2) as osb, \
             tc.tile_pool(name="pso", bufs=1, space="PSUM") as pso, \
             tc.tile_pool(name="ccdram", bufs=1, space="DRAM") as ccdram:
            # asymmetric RS split: g0 = oc0 only (fires early, hides the
            # collective turnaround); g1 = oc1-3 (input ready ~ at RS1 end)
            CCW = (512, 1536)
            cc_in = [ccdram.tile([L, CCW[g_]], BF16, tag=f"ccin{g_}",
                                 name=f"ccin{g_}") for g_ in range(2)]
            cc_out = [ccdram.tile([L // 2, CCW[g_]], BF16, tag=f"ccout{g_}",
                                  name=f"ccout{g_}") for g_ in range(2)]
            for oc in range(4):
                g_ = 0 if oc == 0 else 1
                col = 0 if oc == 0 else (oc - 1) * 512
                pso_t = [pso.tile([P, 512], F32, tag=f"po{b_}",
                                  name=f"pso{oc}_{b_}") for b_ in range(LB)]
                ot = osb.tile([P, LB, 512], BF16, tag="ot", name=f"ot{oc}")
                for b_ in range(LB):   # b_-outer: evict+send as soon as done
                    for ic in range(8):
                        nc.tensor.matmul(
                            pso_t[b_][:], AT[ic][:, b_ * P:(b_ + 1) * P],
                            WOB[:, ic, oc * 512:(oc + 1) * 512],
                            start=(ic == 0), stop=(ic == 7))
                    nc.scalar.copy(ot[:, b_, :], pso_t[b_][:])
                    nc.sync.dma_start(
                        bass.AP(tensor=cc_in[g_].tensor,
                                offset=(cc_in[g_].offset + b_ * P * CCW[g_]
                                        + col),
                                ap=[[CCW[g_], P], [1, 512]]),
                        ot[:, b_, :])
                if oc in (0, 3) and with_collective:
                    # collective issued now (Pool queue, fires on input-ready)
                    nc.gpsimd.collective_compute(
                        "ReduceScatter", ALU.add,
                        replica_groups=[[0, 1], [2, 3], [4, 5], [6, 7]],
                        ins=[cc_in[g_].opt()], outs=[cc_out[g_].opt()])
            for g_ in range(2):
                src_dram = cc_out[g_] if with_collective else cc_in[g_]
                w = CCW[g_]
                # bf16 -> f32 via SBUF bounce, 128-row pipelined chunks
                for r0 in (0, 128, 256, 384):
                    rn = 64 if r0 == 384 else P
                    yb = osb.tile([P, 1536], BF16, tag="yb",
                                  name=f"yb{g_}_{r0}")
                    nc.sync.dma_start(
                        yb[0:rn, 0:w],
                        bass.AP(tensor=src_dram.tensor,
                                offset=src_dram.offset + r0 * w,
                                ap=[[w, rn], [1, w]]))
                    yf = osb.tile([P, 1536], F32, tag="yf",
                                  name=f"yf{g_}_{r0}")
                    nc.scalar.copy(yf[0:rn, 0:w], yb[0:rn, 0:w])
                    nc.sync.dma_start(
                        bass.AP(tensor=y.tensor,
                                offset=(y.offset + r0 * D
                                        + (0 if g_ == 0 else 512)),
                                ap=[[D, rn], [1, w]]),
                        yf[0:rn, 0:w])


# ---------------------------------------------------------------- host side
_ROPE_PERM = np.concatenate([
    np.arange(0, 32, 2), np.arange(1, 32, 2),
    np.arange(32, 64, 2), np.arange(33, 64, 2)])


def make_in_maps(x, wq, wk, wv, wo, temporal_pos, structural_pos):
    import ml_dtypes
    bf16 = ml_dtypes.bfloat16
    x = np.asarray(x, dtype=np.float32)
    wq = np.asarray(wq, dtype=np.float32)
    wk = np.asarray(wk, dtype=np.float32)
    wv = np.asarray(wv, dtype=np.float32)
    wo = np.asarray(wo, dtype=np.float32)
    pt = np.asarray(temporal_pos).astype(np.float64)
    ps = np.asarray(structural_pos).astype(np.float64)
    inv = 1.0 / (10000.0 ** (np.arange(16) / 16.0))
    ct, st = np.cos(pt[:, None] * inv).T, np.sin(pt[:, None] * inv).T
    cs, ss = np.cos(ps[:, None] * inv).T, np.sin(ps[:, None] * inv).T
    Ct = np.concatenate([ct, ct, cs, cs] * 2).astype(bf16)     # [128, 896]
    St = np.concatenate([-st, st, -ss, ss] * 2).astype(bf16)

    wq_p = wq.reshape(HQ, HD, D)[:, _ROPE_PERM, :].reshape(D, D)
    wk_p = wk.reshape(HKV, HD, D)[:, _ROPE_PERM, :].reshape(HKV * HD, D)
    wqT = np.ascontiguousarray(wq_p.T).astype(bf16)   # [D, D]
    wkT = np.ascontiguousarray(wk_p.T).astype(bf16)   # [D, 512]
    wvT = np.ascontiguousarray(wv.T).astype(bf16)     # [D, 512]
    woT = np.ascontiguousarray(wo.T).astype(bf16)     # [D, D]

    in_maps = []
    for c in range(NCORES):
        b, g = divmod(c, 2)
        wkv = np.concatenate([wkT[:, g * KVO:(g + 1) * KVO],
                              wvT[:, g * KVO:(g + 1) * KVO]], axis=1)
        in_maps.append({
            "xT": np.ascontiguousarray(x[b].T).astype(bf16),
            "wqT": np.ascontiguousarray(wqT[:, g * GO:(g + 1) * GO]),
            "wkvT": np.ascontiguousarray(wkv),
            "woT": np.ascontiguousarray(woT[g * GO:(g + 1) * GO, :]),
            "Ct": Ct,
            "St": St,
        })
    return in_maps


def kernel(x, wq, wk, wv, wo, temporal_pos, structural_pos, _trace=False):
    nc = build_nc(with_collective=True)
    in_maps = make_in_maps(x, wq, wk, wv, wo, temporal_pos, structural_pos)
    res = bass_utils.run_bass_kernel_spmd(
        nc, in_maps, core_ids=list(range(NCORES)), trace=_trace)
    out = np.stack([
        np.concatenate([res.results[2 * b]["y"], res.results[2 * b + 1]["y"]],
                       axis=0) for b in range(B)])
    kernel.last_result = res
    return out.astype(np.float32)
